# revision 47
# baseline (speedup 1.0000x reference)
"""Trainium2 Bass kernel for a fused multi-head attention block.

Reference computation (B=4, T=2048, D=1152, H=8, HD=144, full rotary):
    q,k,v = x@Wq.T, x@Wk.T, x@Wv.T   (per head)
    q,k   = rope(q, k, cos, sin)
    o     = softmax(q k^T / sqrt(HD)) v
    out   = o @ Wo.T

Sharding (8 cores): core c = (batch b = c//2, head-group hg = c%2).
Each core computes 4 heads of one batch and a partial output
out_part = o_local @ Wo[:, hg_cols].T ; host sums the two partials per batch.

Per-core structure (v10 — rebuilt from trace analysis of v2, 463us ->
370us on the same box; the binding constraint throughout is PSUM's 8
banks):
  * Phase A (projections+rope+transposes, ~131us): ONE loop; per
    (n t-tile, k d-chunk) FOUR matmuls [q-a 512 | k-a 512 | b' 192 |
    v-a 512] share the x-chunk stationary (b' packs q-b|k-b|v-b 16-dim
    tails of all 4 heads).  rope reads the projection PSUM directly:
    4 DVE muls with a host-precomputed sign-folded/permuted sin table
    (snP) make every combine a plain add, run on GpSimd (otherwise
    idle) to keep DVE from backpressuring the PSUM pool.  v copies out
    on Scalar.  9 PE transposes per n-iteration (4 q, 4 k, 1 shared
    q/k-b block), emitted one-per-k-chunk of the NEXT iteration so the
    single tp PSUM buffer never stalls the PE; copies alternate
    Scalar/DVE.  PSUM: ps_q 2 + ps_k 2 + ps_b 2 + ps_v 1 + tp 1 = 8.
    Warmup matmuls run off a memset tile (no DMA dep) so HAM is warm
    before the first real matmul; the b-row replica tiles (4 row-group
    copies for the score tails) are pre-zeroed on GpSimd and filled by
    16-row DMAs from the transposed qkbT tile.
  * Phase B (attention, ~157us): 512-wide q items (qb, h, kt), flat
    software pipeline, PV lags scores by LOOK=3.  Per item: one K=32
    b-tail + one K=128 main (stop) + exp + 4 PV matmuls (N=145, the
    +1 col = softmax denominator via ones column of v).  The 4 tails
    of each kt-quad are issued together on the 4 distinct PE
    row-groups -> concurrent in the array (~1/4 wall cost); their sps
    allocations (6 bufs) only clear once the exp two items back has
    drained — the exp split is sized so neither engine backs up.
    exp: cols 0:256 exact ACT Exp on Scalar; cols 256:512 on DVE via a
    Schraudolph bitcast exp (one tensor_scalar op: int16(x*a+b) bits
    viewed as bf16, ~1.5% rms rel err on those columns; end-to-end
    rel err 1.2e-2 vs the 2e-2 gate, both engines ~450ns/item).
    Accumulators pack (ql0,ql1)|(ql2,ql3) in two single-buffered
    banks; at head boundaries the batched reciprocal + normalizes run
    at high priority, O1 norms on DVE / O2 norms on Scalar, because
    the next head's bank-clearing PV waits on both norms of its bank.
    PSUM: sps 6 + O1 1 + O2 1 = 8.
  * Phase C (final projection, ~44us): o normalized straight to bf16,
    PE transpose, then k-outer/j3-inner matmuls sharing the o^T
    stationary 1:3.  Wo tiles are DMAed at phase-B start; fout is
    10-deep so the out-DMA (one ~200KB descriptor per [128,384] block)
    never blocks the PSUM->SBUF copies.
  * dtypes: all matmuls bf16 (f32 accum in PSUM); output f32.
"""

import numpy as np

B, T, D, H = 4, 2048, 1152, 8
HL = 4              # heads per core
HD = 144            # head dim
DV = HL * HD        # 576, v/o width
WB = 1728           # merged projection width: q-a 512 | k-a 512 | b' 192 | v-a 512
NT = T // 128       # 16 t-tiles
KC = D // 128       # 9 contraction chunks
SCALE = float(HD) ** -0.5
NCORES = 8

_NC_CACHE = {}


def _build(debug=False):
    import concourse.bacc as bacc
    import concourse.mybir as mybir
    from concourse.tile import TileContext

    dt = mybir.dt
    f32, bf16 = dt.float32, dt.bfloat16
    AF = mybir.ActivationFunctionType

    nc = bacc.Bacc(
        "TRN2",
        target_bir_lowering=False,
        debug=debug,
        enable_asserts=False,
        num_devices=NCORES,
    )

    xT = nc.declare_dram_parameter("xT", [D, T], bf16, isOutput=False)
    wbT = nc.declare_dram_parameter("wbT", [D, WB], bf16, isOutput=False)
    woT = nc.declare_dram_parameter("woT", [DV, D], bf16, isOutput=False)
    cosN = nc.declare_dram_parameter("cosN", [T, HD], bf16, isOutput=False)
    snPN = nc.declare_dram_parameter("snPN", [T, HD], bf16, isOutput=False)
    identB = nc.declare_dram_parameter("identB", [128, 128], bf16, isOutput=False)
    out = nc.declare_dram_parameter("out", [T, D], f32, isOutput=True)

    with TileContext(nc) as tc:
        with tc.tile_pool(name="persist", bufs=1) as P0:
            ident_bf = P0.tile([128, 128], bf16, name="ident_bf", tag="ident_bf")
            nc.sync.dma_start(ident_bf[:], identB[:])

            qTa = [
                P0.tile([128, T], bf16, name=f"qTa{h}", tag=f"qTa{h}")
                for h in range(HL)
            ]
            kTa = [
                P0.tile([128, T], bf16, name=f"kTa{h}", tag=f"kTa{h}")
                for h in range(HL)
            ]
            # b-block rows (transposed): partitions 0:64 = q-b (16 per head,
            # h-major), 64:128 = k-b
            qkbT = P0.tile([128, T], bf16, name="qkbT", tag="qkbT")
            # per-head replicas of the b-block rows at all four 32-row groups,
            # so four score-tail K=32 matmuls can issue to distinct PE
            # row-groups and overlap in the array
            qTBr = [
                P0.tile([128, T], bf16, name=f"qTBr{h}", tag=f"qTBr{h}")
                for h in range(HL)
            ]
            kTBr = [
                P0.tile([128, T], bf16, name=f"kTBr{h}", tag=f"kTBr{h}")
                for h in range(HL)
            ]
            vt = [
                P0.tile([128, HL * (HD + 1)], bf16, name=f"v{t}", tag=f"v{t}")
                for t in range(NT)
            ]

            # ---------------- Phase A: projections + rope + transposes -----
            # v4: ONE loop; per (n,k) FOUR matmuls [q 512 | k 512 | b' 192 |
            # v 512] share the x-chunk stationary (b' = q-b|k-b|v-b packed).
            # PSUM: ps_q 2 + ps_k 2 + ps_b 2 + ps_v 1 + tp 1 = 8 banks.
            # The 9 transposes of iteration n-1 are emitted one-per-k-chunk
            # inside iteration n so the single tp buffer never stalls the PE.
            with tc.tile_pool(name="pa", bufs=1) as pa:
                # pre-zero the replica tiles: the b-row replication below
                # only fills 16 of each 32-row group (the other 16 must be
                # zero for the K=32 tails); GpSimd is idle this early
                # warmup stationary via memset (no DMA dependency) so the
                # HAM-warming dummy matmuls start during runtime preamble;
                # must precede the 8 big replica memsets in the GpSimd queue
                warm_stat = pa.tile([128, 128], bf16, name="warm_stat", tag="warm_stat")
                nc.gpsimd.memset(warm_stat[:], 0.0)
                for hh in range(HL):
                    nc.gpsimd.memset(qTBr[hh][:], 0.0)
                    nc.gpsimd.memset(kTBr[hh][:], 0.0)

                xbig = pa.tile([128, KC * T], bf16, name="xbig", tag="xbig")
                x3 = xbig.rearrange("p (c t) -> p c t", c=KC)
                xs = xT.rearrange("(c p) t -> p c t", p=128)
                xtiles = [x3[:, k] for k in range(KC)]
                cos_sb = pa.tile([128, NT * HD], bf16, name="cos_sb", tag="cos_sb")
                snp_sb = pa.tile([128, NT * HD], bf16, name="snp_sb", tag="snp_sb")
                wbig = pa.tile([128, KC * WB], bf16, name="wbig", tag="wbig")
                wb3 = wbig.rearrange("p (c e) -> p c e", c=KC)
                wb_tiles = [wb3[:, k] for k in range(KC)]
                # first wave at chunk granularity (wb_k + x[k, piece0]
                # interleaved) so matmul (n=0,k) unblocks as pair k lands;
                # later x pieces are single descriptors (Sync-engine issue
                # is ~630 ns per descriptor)
                NP = 8
                PW = T // NP
                wbs = wbT.rearrange("(c p) e -> p c e", p=128)
                # x piece first (smaller, needed with wb chunk cols 0:512
                # for the first matmul); wb chunks split in two descriptors
                # so the q-projection unblocks after ~130KB
                for k in range(KC):
                    nc.sync.dma_start(
                        x3[:, k, 0:PW], xs[:, k, 0:PW]
                    )
                    nc.sync.dma_start(wb3[:, k, 0:512], wbs[:, k, 0:512])
                    nc.sync.dma_start(wb3[:, k, 512:WB], wbs[:, k, 512:WB])
                for p in range(1, NP):
                    nc.sync.dma_start(
                        x3[:, :, p * PW : (p + 1) * PW],
                        xs[:, :, p * PW : (p + 1) * PW],
                    )
                nc.sync.dma_start(
                    cos_sb.rearrange("p (n r) -> p n r", n=NT),
                    cosN.rearrange("(n p) r -> p n r", p=128),
                )
                nc.sync.dma_start(
                    snp_sb.rearrange("p (n r) -> p n r", n=NT),
                    snPN.rearrange("(n p) r -> p n r", p=128),
                )

                def trig3(sb, n):
                    # [128, 144] row block for t-tile n, broadcast over 4 heads
                    return (
                        sb[:, n * HD : (n + 1) * HD]
                        .rearrange("p (o r) -> p o r", o=1)
                        .to_broadcast([128, HL, HD])
                    )

                def rope_b(ps_bq, m1, m2, cos3, snp3):
                    """The two b-dim muls -- issued for q AND k before
                    anything else so ps_b (bufs=1) frees early."""
                    m1b = m1[:, 512:576].rearrange("p (h e) -> p h e", h=HL)
                    m2b = m2[:, 512:576].rearrange("p (h e) -> p h e", h=HL)
                    nc.vector.tensor_mul(m1b[:], ps_bq[:], cos3[:, :, 128:144])
                    nc.vector.tensor_mul(m2b[:], ps_bq[:], snp3[:, :, 128:144])

                def rope_a(ps_a, m1, m2, qtl, ob3, cos3, snp3):
                    """ps_a [128,512] f32 (4 a-blocks) -> qtl [128,512] bf16
                    (a) and ob3 [128,4,16] bf16 (b) with rotary applied.

                    m1[j] = q[j]*cos[j]; m2[j] = q[j]*snP[j] where
                    snP[i] = sin[(i+72)%144] * (+1 if i<72 else -1), so every
                    combine is a plain add: out[j] = m1[j] + m2[(j+72)%144].
                    """
                    pa3 = ps_a.rearrange("p (h e) -> p h e", h=HL)
                    m1a = m1[:, 0:512].rearrange("p (h e) -> p h e", h=HL)
                    m1b = m1[:, 512:576].rearrange("p (h e) -> p h e", h=HL)
                    m2a = m2[:, 0:512].rearrange("p (h e) -> p h e", h=HL)
                    m2b = m2[:, 512:576].rearrange("p (h e) -> p h e", h=HL)
                    v = nc.vector
                    v.tensor_mul(m1a[:], pa3[:], cos3[:, :, 0:128])
                    v.tensor_mul(m2a[:], pa3[:], snp3[:, :, 0:128])
                    oa = qtl.rearrange("p (h e) -> p h e", h=HL)
                    # all-bf16 SBUF operands; run the combines on GpSimd
                    # (idle otherwise) so DVE only does the 4 PSUM-read muls
                    g = nc.gpsimd
                    g.tensor_add(oa[:, :, 0:56], m1a[:, :, 0:56], m2a[:, :, 72:128])
                    g.tensor_add(oa[:, :, 56:72], m1a[:, :, 56:72], m2b[:, :, 0:16])
                    g.tensor_add(oa[:, :, 72:128], m1a[:, :, 72:128], m2a[:, :, 0:56])
                    g.tensor_add(ob3[:], m1b[:], m2a[:, :, 56:72])

                with tc.tile_pool(name="paqps", bufs=1, space="PSUM") as paqps:
                    warm_ps = paqps.tile(
                        [128, 512], f32, name="warm_ps", tag="pv", bufs=1
                    )
                    with tc.high_priority():
                        for _ in range(10):
                            nc.tensor.matmul(
                                warm_ps[:, 0:128], warm_stat[:], warm_stat[:],
                                start=True, stop=True,
                            )

                    def one_transpose(pend, j):
                        n, qtl, ktl, qkb = pend
                        tp = paqps.tile(
                            [128, 128], bf16, name="tp", tag="tp", bufs=1
                        )
                        if j < 4:
                            src, dst = qtl[:, 128 * j : 128 * (j + 1)], qTa[j]
                        elif j < 8:
                            src, dst = ktl[:, 128 * (j - 4) : 128 * (j - 3)], kTa[j - 4]
                        else:
                            src, dst = qkb[:], qkbT
                        nc.tensor.transpose(tp[:], src, ident_bf[:])
                        if j % 2:
                            nc.scalar.copy(dst[:, n * 128 : (n + 1) * 128], tp[:])
                        else:
                            nc.vector.tensor_copy(
                                dst[:, n * 128 : (n + 1) * 128], tp[:]
                            )

                    pend = None
                    for n in range(NT):
                        ps_q = paqps.tile(
                            [128, 512], f32, name="ps_q", tag="psq", bufs=2
                        )
                        ps_k = paqps.tile(
                            [128, 512], f32, name="ps_k", tag="psk", bufs=2
                        )
                        ps_b = paqps.tile(
                            [128, 192], f32, name="ps_b", tag="psb", bufs=2
                        )
                        ps_v = paqps.tile(
                            [128, 512], f32, name="ps_v", tag="pv", bufs=1
                        )
                        for k in range(KC):
                            st, sp = k == 0, k == KC - 1
                            lhs = xtiles[k][:, n * 128 : (n + 1) * 128]
                            nc.tensor.matmul(
                                ps_q[:], lhs, wb_tiles[k][:, 0:512],
                                start=st, stop=sp,
                            )
                            nc.tensor.matmul(
                                ps_k[:], lhs, wb_tiles[k][:, 512:1024],
                                start=st, stop=sp,
                            )
                            nc.tensor.matmul(
                                ps_b[:], lhs, wb_tiles[k][:, 1024:1216],
                                start=st, stop=sp,
                            )
                            nc.tensor.matmul(
                                ps_v[:], lhs, wb_tiles[k][:, 1216:WB],
                                start=st, stop=sp,
                            )
                            if pend is not None:
                                one_transpose(pend, k)
                        v3 = vt[n].rearrange("p (h e) -> p h e", h=HL)
                        c3, s3 = trig3(cos_sb, n), trig3(snp_sb, n)
                        # scalar engine: DVE is saturated by rope in the
                        # steady state and was backpressuring the v matmuls
                        nc.scalar.copy(
                            v3[:, :, 128:HD],
                            ps_b[:, 128:192].rearrange("p (h e) -> p h e", h=HL),
                        )
                        nc.scalar.copy(
                            v3[:, :, 0:128],
                            ps_v.rearrange("p (h e) -> p h e", h=HL),
                        )
                        nc.gpsimd.memset(v3[:, :, HD : HD + 1], 1.0)
                        qtl = pa.tile([128, 512], bf16, name="qtl", tag="qtl", bufs=2)
                        ktl = pa.tile([128, 512], bf16, name="ktl", tag="ktl", bufs=2)
                        qkb = pa.tile([128, 128], bf16, name="qkb", tag="qkb", bufs=2)
                        m1q = pa.tile([128, 576], bf16, name="m1q", tag="m1q", bufs=2)
                        m2q = pa.tile([128, 576], bf16, name="m2q", tag="m2q", bufs=2)
                        m1k = pa.tile([128, 576], bf16, name="m1k", tag="m1k", bufs=2)
                        m2k = pa.tile([128, 576], bf16, name="m2k", tag="m2k", bufs=2)
                        rope_b(
                            ps_b[:, 0:64].rearrange("p (h e) -> p h e", h=HL),
                            m1q, m2q, c3, s3,
                        )
                        rope_a(
                            ps_q, m1q, m2q, qtl,
                            qkb[:, 0:64].rearrange("p (h e) -> p h e", h=HL),
                            c3, s3,
                        )
                        rope_b(
                            ps_b[:, 64:128].rearrange("p (h e) -> p h e", h=HL),
                            m1k, m2k, c3, s3,
                        )
                        rope_a(
                            ps_k, m1k, m2k, ktl,
                            qkb[:, 64:128].rearrange("p (h e) -> p h e", h=HL),
                            c3, s3,
                        )
                        pend = (n, qtl, ktl, qkb)
                    for j in range(9):
                        one_transpose(pend, j)

                # replicate the b-rows to all four 32-row groups (16 data
                # rows each; the other 16 rows were pre-zeroed), in two
                # column halves so the first half's DMAs issue as soon as
                # transposes n<=7 are done (subtile deps)
                for half in range(2):
                    cl, cr = half * 1024, (half + 1) * 1024
                    for hh in range(HL):
                        for j in range(4):
                            nc.sync.dma_start(
                                qTBr[hh][32 * j : 32 * j + 16, cl:cr],
                                qkbT[16 * hh : 16 * hh + 16, cl:cr],
                            )
                            nc.sync.dma_start(
                                kTBr[hh][32 * j : 32 * j + 16, cl:cr],
                                qkbT[64 + 16 * hh : 64 + 16 * hh + 16, cl:cr],
                            )

            # ---------------- Phase B: attention --------------------------
            # v3: 512-wide q items (qb, h, kt).  Per item: one K=32 b-tail
            # (4 consecutive kts' tails issued together on the 4 distinct PE
            # row-groups -> concurrent in the array, ~1/4 the wall cost) +
            # one K=128 main (stop) + exp + 4 PV matmuls.  The exp is split:
            # cols 0:EXS on Scalar (exact ACT Exp), cols EXS:512 on DVE via a
            # Schraudolph bitcast (int16(x*a+b) viewed as bf16, one
            # tensor_scalar op) so neither engine paces the ~660ns/item PE
            # stream.  PSUM: sps 5x1 bank + O1 (ql 0-2) 2x1 + O2 (ql 3 +
            # denom) 1x1 = 8 banks.  O2 is normalized first at head
            # boundaries; the next head's ql3 PV arrives ~760ns later.
            with tc.tile_pool(name="pb", bufs=1) as pb:
                ot = [
                    pb.tile([128, DV], bf16, name=f"o{t}", tag=f"o{t}")
                    for t in range(NT)
                ]
                # phase C's Wo chunks: issue the DMAs now so they stream in
                # during phase B instead of gating the first final matmuls
                wo_tiles = []
                for k in range(5):
                    rows = 128 if k < 4 else 64
                    wot_ = pb.tile([128, D], bf16, name=f"wo{k}", tag=f"wo{k}")
                    nc.sync.dma_start(
                        wot_[0:rows, :], woT[k * 128 : k * 128 + rows, :]
                    )
                    wo_tiles.append(wot_)
                with tc.tile_pool(name="pbps", bufs=1, space="PSUM") as pbps:
                    HD1 = HD + 1
                    QB = 4              # 512-wide q blocks
                    NITEM = QB * HL * NT
                    LOOK = 3            # PV lags scores by 3 items
                    EXS = 256           # scalar-exp columns per 512
                    # Schraudolph bf16 exp: bits = int16(x*EXA + EXB)
                    EXA = SCALE * (2.0 ** 7) / float(np.log(2.0))
                    EXB = 127.0 * 128.0 - 7.5
                    i16 = dt.int16
                    Alu = mybir.AluOpType

                    def decode(idx):
                        qb, rem = divmod(idx, HL * NT)
                        h, kt = divmod(rem, NT)
                        return qb, h, kt

                    sps_pend = {}
                    e_of = {}

                    def stage_tails(s):
                        # 4 consecutive kts' b-tails -> 4 distinct row-groups.
                        # Consecutive kts PAIR into one 2-bank sps tile so the
                        # exp runs as ONE strided op per engine per pair
                        # (amortizes the ~180ns ACT fixed cost) and the
                        # 3-deep pair rotation has 4 items of exp slack.
                        qb, h, kt0 = decode(s)
                        for j in (0, 2):
                            sps = pbps.tile(
                                [128, 1024], f32, name="sps", tag="sc", bufs=3
                            )
                            sps_pend[s + j] = (sps, 0)
                            sps_pend[s + j + 1] = (sps, 1)
                        for j in range(4):
                            kt = kt0 + j
                            rg = kt % 4
                            sps, half = sps_pend[s + j]
                            nc.tensor.matmul(
                                sps[:, 512 * half : 512 * (half + 1)],
                                kTBr[h][
                                    32 * rg : 32 * rg + 32,
                                    kt * 128 : (kt + 1) * 128,
                                ],
                                qTBr[h][
                                    32 * rg : 32 * rg + 32,
                                    qb * 512 : (qb + 1) * 512,
                                ],
                                start=True,
                                stop=False,
                                tile_position=(32 * rg, 0),
                            )

                    def stage_main(s):
                        qb, h, kt = decode(s)
                        sps, half = sps_pend.pop(s)
                        nc.tensor.matmul(
                            sps[:, 512 * half : 512 * (half + 1)],
                            kTa[h][:, kt * 128 : (kt + 1) * 128],
                            qTa[h][:, qb * 512 : (qb + 1) * 512],
                            start=False,
                            stop=True,
                        )
                        if half == 1:
                            # pair complete: one exp op per engine covers
                            # both items via a [128, 2, c] strided view
                            E = pb.tile(
                                [128, 1024], bf16, name="E", tag="E", bufs=3
                            )
                            sp2 = sps.rearrange("p (t c) -> p t c", t=2)
                            e2 = E.rearrange("p (t c) -> p t c", t=2)
                            nc.scalar.activation(
                                e2[:, :, 0:EXS], sp2[:, :, 0:EXS],
                                AF.Exp, scale=SCALE,
                            )
                            nc.vector.tensor_scalar(
                                E.bitcast(i16).rearrange(
                                    "p (t c) -> p t c", t=2
                                )[:, :, EXS:512],
                                sp2[:, :, EXS:512],
                                EXA,
                                EXB,
                                Alu.mult,
                                Alu.add,
                            )
                            e_of[s // 2] = E

                    o_ps = None
                    for s in range(NITEM + LOOK):
                        if s < NITEM:
                            if s % 4 == 0:
                                stage_tails(s)
                            stage_main(s)
                        if s < LOOK:
                            continue
                        idx = s - LOOK
                        qb, h, kt = decode(idx)
                        if kt == 0:
                            # (ql0,ql1) and (ql2,ql3) pair up in two banks:
                            # a start=True PV clears its whole bank, so each
                            # new head's PV ql waits on only TWO normalizes
                            # of the previous head, not three
                            O1 = pbps.tile(
                                [128, 2 * HD1], f32, name="O1", tag="O1", bufs=1
                            )
                            O2 = pbps.tile(
                                [128, 2 * HD1], f32, name="O2", tag="O2", bufs=1
                            )
                            o_ps = [
                                O1[:, 0:HD1], O1[:, HD1 : 2 * HD1],
                                O2[:, 0:HD1], O2[:, HD1 : 2 * HD1],
                            ]
                        E = e_of[idx // 2]
                        if idx % 2 == 1:
                            del e_of[idx // 2]
                        ecol = (idx % 2) * 512
                        for ql in range(4):
                            st = kt == 0 and ql in (0, 2)
                            sp = kt == NT - 1 and ql in (1, 3)
                            nc.tensor.matmul(
                                o_ps[ql][:],
                                E[:, ecol + ql * 128 : ecol + (ql + 1) * 128],
                                vt[kt][:, HD1 * h : HD1 * (h + 1)],
                                start=st,
                                stop=sp,
                            )
                        if kt == NT - 1:
                            # accumulators are single-buffered: the next
                            # head's bank-clearing PV stalls on BOTH norms of
                            # that bank.  Batch the reciprocals (one [128,2]
                            # op per bank), then O1's norms on DVE and O2's
                            # on Scalar, all at high priority so they beat
                            # the queued exps of the in-flight score stream.
                            with tc.high_priority():
                                dsts = [
                                    ot[4 * qb + ql][:, HD * h : HD * (h + 1)]
                                    for ql in range(4)
                                ]
                                rA = pb.tile([128, 2], f32, name="rA", tag="rA", bufs=2)
                                rB = pb.tile([128, 2], f32, name="rB", tag="rB", bufs=2)
                                nc.vector.reciprocal_approx_fast(
                                    rA.rearrange("p (q o) -> p q o", q=2),
                                    O1.rearrange("p (q e) -> p q e", q=2)[
                                        :, :, HD : HD + 1
                                    ],
                                )
                                nc.vector.reciprocal_approx_fast(
                                    rB.rearrange("p (q o) -> p q o", q=2),
                                    O2.rearrange("p (q e) -> p q e", q=2)[
                                        :, :, HD : HD + 1
                                    ],
                                )
                                nc.scalar.activation(
                                    dsts[2], o_ps[2][:, 0:HD], AF.Copy,
                                    scale=rB[:, 0:1],
                                )
                                nc.scalar.activation(
                                    dsts[3], o_ps[3][:, 0:HD], AF.Copy,
                                    scale=rB[:, 1:2],
                                )
                                nc.vector.tensor_scalar_mul(
                                    dsts[0], o_ps[0][:, 0:HD], rA[:, 0:1]
                                )
                                nc.vector.tensor_scalar_mul(
                                    dsts[1], o_ps[1][:, 0:HD], rA[:, 1:2]
                                )

                # ---------------- Phase C: o^T + final projection ----------
                oTa = [
                    pb.tile([128, T], bf16, name=f"oTa{j}", tag=f"oTa{j}")
                    for j in range(4)
                ]
                oTb = pb.tile([64, T], bf16, name="oTb", tag="oTb")
                with tc.tile_pool(name="pcps", bufs=1, space="PSUM") as pcps:

                    def o_transp(t):
                        for j in range(4):
                            tp = pcps.tile(
                                [128, 128], bf16, name="tpo", tag="otp", bufs=3
                            )
                            nc.tensor.transpose(
                                tp[:],
                                ot[t][:, 128 * j : 128 * (j + 1)],
                                ident_bf[:],
                            )
                            nc.vector.tensor_copy(
                                oTa[j][:, t * 128 : (t + 1) * 128], tp[:]
                            )
                        tpb = pcps.tile([64, 128], bf16, name="tpb", tag="otp", bufs=3)
                        nc.tensor.transpose(
                            tpb[:],
                            ot[t][:, 512:DV],
                            ident_bf[:],
                        )
                        nc.vector.tensor_copy(
                            oTb[:, t * 128 : (t + 1) * 128], tpb[:]
                        )

                    def final(t):
                        fps = [
                            pcps.tile(
                                [128, 384], f32, name=f"fps{j3}", tag=f"f{j3}",
                                bufs=(2 if j3 < 2 else 1),
                            )
                            for j3 in range(3)
                        ]
                        # k-outer / j3-inner: the 3 matmuls of each k share
                        # the o^T stationary, hiding its LDWEIGHTS
                        for k in range(5):
                            rows = 128 if k < 4 else 64
                            lhsT = (
                                oTa[k][:, t * 128 : (t + 1) * 128]
                                if k < 4
                                else oTb[:, t * 128 : (t + 1) * 128]
                            )
                            for j3 in range(3):
                                nc.tensor.matmul(
                                    fps[j3][:],
                                    lhsT,
                                    wo_tiles[k][0:rows, 384 * j3 : 384 * (j3 + 1)],
                                    start=(k == 0),
                                    stop=(k == 4),
                                )
                        for j3 in range(3):
                            fout = pb.tile(
                                [128, 384], f32, name="fout", tag="fout", bufs=10
                            )
                            if (t * 3 + j3) % 2 == 1:
                                nc.vector.tensor_copy(fout[:], fps[j3][:])
                            else:
                                nc.scalar.copy(fout[:], fps[j3][:])
                            nc.sync.dma_start(
                                out[
                                    t * 128 : (t + 1) * 128,
                                    384 * j3 : 384 * (j3 + 1),
                                ],
                                fout[:],
                            )

                    o_transp(0)
                    for t in range(NT):
                        if t + 1 < NT:
                            o_transp(t + 1)
                        final(t)

    nc.compile()
    return nc


def get_nc(debug=False):
    key = bool(debug)
    if key not in _NC_CACHE:
        _NC_CACHE[key] = _build(debug)
    return _NC_CACHE[key]


def make_in_maps(x, cos, sin, Wq, Wk, Wv, Wo):
    import ml_dtypes

    x = np.asarray(x, np.float32)
    cos = np.asarray(cos, np.float32)
    sin = np.asarray(sin, np.float32)
    Wq, Wk, Wv, Wo = (np.asarray(w, np.float32) for w in (Wq, Wk, Wv, Wo))
    cos_bf = cos.astype(ml_dtypes.bfloat16)
    # sign-folded, partner-permuted sin: snP[t,i] = sin[t,(i+72)%144] * s,
    # s = +1 for i<72, -1 for i>=72; makes every rope combine a plain add
    snp = sin[:, (np.arange(HD) + 72) % HD].copy()
    snp[:, 72:] *= -1.0
    snp_bf = snp.astype(ml_dtypes.bfloat16)

    in_maps = []
    for c in range(NCORES):
        b, hg = divmod(c, 2)
        heads = [HL * hg + i for i in range(HL)]

        def w_merged(Wq_, Wk_, Wv_):
            # rows: [q-a 4x128 | k-a 4x128 | q-b 4x16 | k-b 4x16 | v-b 4x16
            #        | v-a 4x128]
            Wsel = np.zeros((WB, D), np.float32)
            for i, g in enumerate(heads):
                a, bb = 144 * g, 144 * g + 128
                Wsel[128 * i : 128 * i + 128] = Wq_[a : a + 128]
                Wsel[512 + 128 * i : 512 + 128 * i + 128] = Wk_[a : a + 128]
                Wsel[1024 + 16 * i : 1024 + 16 * i + 16] = Wq_[bb : bb + 16]
                Wsel[1088 + 16 * i : 1088 + 16 * i + 16] = Wk_[bb : bb + 16]
                Wsel[1152 + 16 * i : 1152 + 16 * i + 16] = Wv_[bb : bb + 16]
                Wsel[1216 + 128 * i : 1216 + 128 * i + 128] = Wv_[a : a + 128]
            return np.ascontiguousarray(Wsel.T)

        wo_sel = np.concatenate([Wo[:, 144 * g : 144 * g + 144] for g in heads], 1)
        in_maps.append(
            {
                "xT": np.ascontiguousarray(x[b].T).astype(ml_dtypes.bfloat16),
                "wbT": w_merged(Wq, Wk, Wv).astype(ml_dtypes.bfloat16),
                "woT": np.ascontiguousarray(wo_sel.T).astype(ml_dtypes.bfloat16),
                "cosN": cos_bf,
                "snPN": snp_bf,
                "identB": np.eye(128, dtype=ml_dtypes.bfloat16),
            }
        )
    return in_maps


def kernel(x, cos, sin, Wq, Wk, Wv, Wo, _trace=False, _trace_kwargs=None):
    from concourse.bass_utils import run_bass_kernel_spmd

    nc = get_nc()
    in_maps = make_in_maps(x, cos, sin, Wq, Wk, Wv, Wo)
    res = run_bass_kernel_spmd(
        nc,
        in_maps,
        list(range(NCORES)),
        trace=_trace,
        **(_trace_kwargs or {}),
    )
    parts = [res.results[c]["out"] for c in range(NCORES)]
    outb = np.stack([parts[2 * b] + parts[2 * b + 1] for b in range(B)])
    if _trace:
        kernel.last_results = res
    return outb.astype(np.float32)



# revision 48
# speedup vs baseline: 1.0690x; 1.0690x over previous
"""Trainium2 Bass kernel for a fused multi-head attention block.

Reference computation (B=4, T=2048, D=1152, H=8, HD=144, full rotary):
    q,k,v = x@Wq.T, x@Wk.T, x@Wv.T   (per head)
    q,k   = rope(q, k, cos, sin)
    o     = softmax(q k^T / sqrt(HD)) v
    out   = o @ Wo.T

Sharding (8 cores): core c = (batch b = c//2, head-group hg = c%2).
Each core computes 4 heads of one batch and a partial output
out_part = o_local @ Wo[:, hg_cols].T ; host sums the two partials per batch.

Per-core structure (v10 — rebuilt from trace analysis of v2, 463us ->
370us on the same box; the binding constraint throughout is PSUM's 8
banks):
  * Phase A (projections+rope+transposes, ~131us): ONE loop; per
    (n t-tile, k d-chunk) FOUR matmuls [q-a 512 | k-a 512 | b' 192 |
    v-a 512] share the x-chunk stationary (b' packs q-b|k-b|v-b 16-dim
    tails of all 4 heads).  rope reads the projection PSUM directly:
    4 DVE muls with a host-precomputed sign-folded/permuted sin table
    (snP) make every combine a plain add, run on GpSimd (otherwise
    idle) to keep DVE from backpressuring the PSUM pool.  v copies out
    on Scalar.  9 PE transposes per n-iteration (4 q, 4 k, 1 shared
    q/k-b block), emitted one-per-k-chunk of the NEXT iteration so the
    single tp PSUM buffer never stalls the PE; copies alternate
    Scalar/DVE.  PSUM: ps_q 2 + ps_k 2 + ps_b 2 + ps_v 1 + tp 1 = 8.
    Warmup matmuls run off a memset tile (no DMA dep) so HAM is warm
    before the first real matmul; the b-row replica tiles (4 row-group
    copies for the score tails) are pre-zeroed on GpSimd and filled by
    16-row DMAs from the transposed qkbT tile.
  * Phase B (attention, ~157us): 512-wide q items (qb, h, kt), flat
    software pipeline, PV lags scores by LOOK=3.  Per item: one K=32
    b-tail + one K=128 main (stop) + exp + 4 PV matmuls (N=145, the
    +1 col = softmax denominator via ones column of v).  The 4 tails
    of each kt-quad are issued together on the 4 distinct PE
    row-groups -> concurrent in the array (~1/4 wall cost); their sps
    allocations (6 bufs) only clear once the exp two items back has
    drained — the exp split is sized so neither engine backs up.
    exp: cols 0:256 exact ACT Exp on Scalar; cols 256:512 on DVE via a
    Schraudolph bitcast exp (one tensor_scalar op: int16(x*a+b) bits
    viewed as bf16, ~1.5% rms rel err on those columns; end-to-end
    rel err 1.2e-2 vs the 2e-2 gate, both engines ~450ns/item).
    Accumulators pack (ql0,ql1)|(ql2,ql3) in two single-buffered
    banks; at head boundaries the batched reciprocal + normalizes run
    at high priority, O1 norms on DVE / O2 norms on Scalar, because
    the next head's bank-clearing PV waits on both norms of its bank.
    PSUM: sps 6 + O1 1 + O2 1 = 8.
  * Phase C (final projection, ~44us): o normalized straight to bf16,
    PE transpose, then k-outer/j3-inner matmuls sharing the o^T
    stationary 1:3.  Wo tiles are DMAed at phase-B start; fout is
    10-deep so the out-DMA (one ~200KB descriptor per [128,384] block)
    never blocks the PSUM->SBUF copies.
  * dtypes: all matmuls bf16 (f32 accum in PSUM); output f32.
"""

import numpy as np

B, T, D, H = 4, 2048, 1152, 8
HL = 4              # heads per core
HD = 144            # head dim
DV = HL * HD        # 576, v/o width
WB = 1728           # merged projection width: q-a 512 | k-a 512 | b' 192 | v-a 512
NT = T // 128       # 16 t-tiles
KC = D // 128       # 9 contraction chunks
SCALE = float(HD) ** -0.5
NCORES = 8

_NC_CACHE = {}


def _build(debug=False):
    import concourse.bacc as bacc
    import concourse.mybir as mybir
    from concourse.tile import TileContext

    dt = mybir.dt
    f32, bf16 = dt.float32, dt.bfloat16
    AF = mybir.ActivationFunctionType

    nc = bacc.Bacc(
        "TRN2",
        target_bir_lowering=False,
        debug=debug,
        enable_asserts=False,
        num_devices=NCORES,
    )

    xT = nc.declare_dram_parameter("xT", [D, T], bf16, isOutput=False)
    wbT = nc.declare_dram_parameter("wbT", [D, WB], bf16, isOutput=False)
    woT = nc.declare_dram_parameter("woT", [DV, D], bf16, isOutput=False)
    cosN = nc.declare_dram_parameter("cosN", [T, HD], bf16, isOutput=False)
    snPN = nc.declare_dram_parameter("snPN", [T, HD], bf16, isOutput=False)
    identB = nc.declare_dram_parameter("identB", [128, 128], bf16, isOutput=False)
    out = nc.declare_dram_parameter("out", [T, D], f32, isOutput=True)

    with TileContext(nc) as tc:
        with tc.tile_pool(name="persist", bufs=1) as P0:
            ident_bf = P0.tile([128, 128], bf16, name="ident_bf", tag="ident_bf")
            nc.sync.dma_start(ident_bf[:], identB[:])

            qTa = [
                P0.tile([128, T], bf16, name=f"qTa{h}", tag=f"qTa{h}")
                for h in range(HL)
            ]
            kTa = [
                P0.tile([128, T], bf16, name=f"kTa{h}", tag=f"kTa{h}")
                for h in range(HL)
            ]
            # b-block rows (transposed): partitions 0:64 = q-b (16 per head,
            # h-major), 64:128 = k-b
            qkbT = P0.tile([128, T], bf16, name="qkbT", tag="qkbT")
            # per-head replicas of the b-block rows at all four 32-row groups,
            # so four score-tail K=32 matmuls can issue to distinct PE
            # row-groups and overlap in the array
            qTBr = [
                P0.tile([128, T], bf16, name=f"qTBr{h}", tag=f"qTBr{h}")
                for h in range(HL)
            ]
            kTBr = [
                P0.tile([128, T], bf16, name=f"kTBr{h}", tag=f"kTBr{h}")
                for h in range(HL)
            ]
            vt = [
                P0.tile([128, HL * (HD + 1)], bf16, name=f"v{t}", tag=f"v{t}")
                for t in range(NT)
            ]

            # ---------------- Phase A: projections + rope + transposes -----
            # v4: ONE loop; per (n,k) FOUR matmuls [q 512 | k 512 | b' 192 |
            # v 512] share the x-chunk stationary (b' = q-b|k-b|v-b packed).
            # PSUM: ps_q 2 + ps_k 2 + ps_b 2 + ps_v 1 + tp 1 = 8 banks.
            # The 9 transposes of iteration n-1 are emitted one-per-k-chunk
            # inside iteration n so the single tp buffer never stalls the PE.
            with tc.tile_pool(name="pa", bufs=1) as pa:
                # pre-zero the replica tiles: the b-row replication below
                # only fills 16 of each 32-row group (the other 16 must be
                # zero for the K=32 tails); GpSimd is idle this early
                # warmup stationary via memset (no DMA dependency) so the
                # HAM-warming dummy matmuls start during runtime preamble;
                # must precede the 8 big replica memsets in the GpSimd queue
                warm_stat = pa.tile([128, 128], bf16, name="warm_stat", tag="warm_stat")
                nc.gpsimd.memset(warm_stat[:], 0.0)
                for hh in range(HL):
                    nc.gpsimd.memset(qTBr[hh][:], 0.0)
                    nc.gpsimd.memset(kTBr[hh][:], 0.0)

                xbig = pa.tile([128, KC * T], bf16, name="xbig", tag="xbig")
                x3 = xbig.rearrange("p (c t) -> p c t", c=KC)
                xs = xT.rearrange("(c p) t -> p c t", p=128)
                xtiles = [x3[:, k] for k in range(KC)]
                cos_sb = pa.tile([128, NT * HD], bf16, name="cos_sb", tag="cos_sb")
                snp_sb = pa.tile([128, NT * HD], bf16, name="snp_sb", tag="snp_sb")
                wbig = pa.tile([128, KC * WB], bf16, name="wbig", tag="wbig")
                wb3 = wbig.rearrange("p (c e) -> p c e", c=KC)
                wb_tiles = [wb3[:, k] for k in range(KC)]
                # first wave at chunk granularity (wb_k + x[k, piece0]
                # interleaved) so matmul (n=0,k) unblocks as pair k lands;
                # later x pieces are single descriptors (Sync-engine issue
                # is ~630 ns per descriptor)
                NP = 8
                PW = T // NP
                wbs = wbT.rearrange("(c p) e -> p c e", p=128)
                # x piece first (smaller, needed with wb chunk cols 0:512
                # for the first matmul); wb chunks split in two descriptors
                # so the q-projection unblocks after ~130KB
                for k in range(KC):
                    nc.sync.dma_start(
                        x3[:, k, 0:PW], xs[:, k, 0:PW]
                    )
                    nc.sync.dma_start(wb3[:, k, 0:512], wbs[:, k, 0:512])
                    nc.sync.dma_start(wb3[:, k, 512:WB], wbs[:, k, 512:WB])
                for p in range(1, NP):
                    nc.sync.dma_start(
                        x3[:, :, p * PW : (p + 1) * PW],
                        xs[:, :, p * PW : (p + 1) * PW],
                    )
                nc.sync.dma_start(
                    cos_sb.rearrange("p (n r) -> p n r", n=NT),
                    cosN.rearrange("(n p) r -> p n r", p=128),
                )
                nc.sync.dma_start(
                    snp_sb.rearrange("p (n r) -> p n r", n=NT),
                    snPN.rearrange("(n p) r -> p n r", p=128),
                )

                def trig3(sb, n):
                    # [128, 144] row block for t-tile n, broadcast over 4 heads
                    return (
                        sb[:, n * HD : (n + 1) * HD]
                        .rearrange("p (o r) -> p o r", o=1)
                        .to_broadcast([128, HL, HD])
                    )

                def rope_b(ps_bq, m1, m2, cos3, snp3):
                    """The two b-dim muls -- issued for q AND k before
                    anything else so ps_b (bufs=1) frees early."""
                    m1b = m1[:, 512:576].rearrange("p (h e) -> p h e", h=HL)
                    m2b = m2[:, 512:576].rearrange("p (h e) -> p h e", h=HL)
                    nc.vector.tensor_mul(m1b[:], ps_bq[:], cos3[:, :, 128:144])
                    nc.vector.tensor_mul(m2b[:], ps_bq[:], snp3[:, :, 128:144])

                def rope_a(ps_a, m1, m2, qtl, ob3, cos3, snp3):
                    """ps_a [128,512] f32 (4 a-blocks) -> qtl [128,512] bf16
                    (a) and ob3 [128,4,16] bf16 (b) with rotary applied.

                    m1[j] = q[j]*cos[j]; m2[j] = q[j]*snP[j] where
                    snP[i] = sin[(i+72)%144] * (+1 if i<72 else -1), so every
                    combine is a plain add: out[j] = m1[j] + m2[(j+72)%144].
                    """
                    pa3 = ps_a.rearrange("p (h e) -> p h e", h=HL)
                    m1a = m1[:, 0:512].rearrange("p (h e) -> p h e", h=HL)
                    m1b = m1[:, 512:576].rearrange("p (h e) -> p h e", h=HL)
                    m2a = m2[:, 0:512].rearrange("p (h e) -> p h e", h=HL)
                    m2b = m2[:, 512:576].rearrange("p (h e) -> p h e", h=HL)
                    v = nc.vector
                    v.tensor_mul(m1a[:], pa3[:], cos3[:, :, 0:128])
                    v.tensor_mul(m2a[:], pa3[:], snp3[:, :, 0:128])
                    oa = qtl.rearrange("p (h e) -> p h e", h=HL)
                    # all-bf16 SBUF operands; run the combines on GpSimd
                    # (idle otherwise) so DVE only does the 4 PSUM-read muls
                    g = nc.gpsimd
                    g.tensor_add(oa[:, :, 0:56], m1a[:, :, 0:56], m2a[:, :, 72:128])
                    g.tensor_add(oa[:, :, 56:72], m1a[:, :, 56:72], m2b[:, :, 0:16])
                    g.tensor_add(oa[:, :, 72:128], m1a[:, :, 72:128], m2a[:, :, 0:56])
                    g.tensor_add(ob3[:], m1b[:], m2a[:, :, 56:72])

                with tc.tile_pool(name="paqps", bufs=1, space="PSUM") as paqps:
                    warm_ps = paqps.tile(
                        [128, 512], f32, name="warm_ps", tag="pv", bufs=1
                    )
                    with tc.high_priority():
                        for _ in range(10):
                            nc.tensor.matmul(
                                warm_ps[:, 0:128], warm_stat[:], warm_stat[:],
                                start=True, stop=True,
                            )

                    def one_transpose(pend, j):
                        n, qtl, ktl, qkb = pend
                        tp = paqps.tile(
                            [128, 128], bf16, name="tp", tag="tp", bufs=1
                        )
                        if j < 4:
                            src, dst = qtl[:, 128 * j : 128 * (j + 1)], qTa[j]
                        elif j < 8:
                            src, dst = ktl[:, 128 * (j - 4) : 128 * (j - 3)], kTa[j - 4]
                        else:
                            src, dst = qkb[:], qkbT
                        nc.tensor.transpose(tp[:], src, ident_bf[:])
                        if j % 2:
                            nc.scalar.copy(dst[:, n * 128 : (n + 1) * 128], tp[:])
                        else:
                            nc.vector.tensor_copy(
                                dst[:, n * 128 : (n + 1) * 128], tp[:]
                            )

                    pend = None
                    for n in range(NT):
                        ps_q = paqps.tile(
                            [128, 512], f32, name="ps_q", tag="psq", bufs=2
                        )
                        ps_k = paqps.tile(
                            [128, 512], f32, name="ps_k", tag="psk", bufs=2
                        )
                        ps_b = paqps.tile(
                            [128, 192], f32, name="ps_b", tag="psb", bufs=2
                        )
                        ps_v = paqps.tile(
                            [128, 512], f32, name="ps_v", tag="pv", bufs=1
                        )
                        for k in range(KC):
                            st, sp = k == 0, k == KC - 1
                            lhs = xtiles[k][:, n * 128 : (n + 1) * 128]
                            nc.tensor.matmul(
                                ps_q[:], lhs, wb_tiles[k][:, 0:512],
                                start=st, stop=sp,
                            )
                            nc.tensor.matmul(
                                ps_k[:], lhs, wb_tiles[k][:, 512:1024],
                                start=st, stop=sp,
                            )
                            nc.tensor.matmul(
                                ps_b[:], lhs, wb_tiles[k][:, 1024:1216],
                                start=st, stop=sp,
                            )
                            nc.tensor.matmul(
                                ps_v[:], lhs, wb_tiles[k][:, 1216:WB],
                                start=st, stop=sp,
                            )
                            if pend is not None:
                                one_transpose(pend, k)
                        v3 = vt[n].rearrange("p (h e) -> p h e", h=HL)
                        c3, s3 = trig3(cos_sb, n), trig3(snp_sb, n)
                        # scalar engine: DVE is saturated by rope in the
                        # steady state and was backpressuring the v matmuls
                        nc.scalar.copy(
                            v3[:, :, 128:HD],
                            ps_b[:, 128:192].rearrange("p (h e) -> p h e", h=HL),
                        )
                        nc.scalar.copy(
                            v3[:, :, 0:128],
                            ps_v.rearrange("p (h e) -> p h e", h=HL),
                        )
                        nc.gpsimd.memset(v3[:, :, HD : HD + 1], 1.0)
                        qtl = pa.tile([128, 512], bf16, name="qtl", tag="qtl", bufs=2)
                        ktl = pa.tile([128, 512], bf16, name="ktl", tag="ktl", bufs=2)
                        qkb = pa.tile([128, 128], bf16, name="qkb", tag="qkb", bufs=2)
                        m1q = pa.tile([128, 576], bf16, name="m1q", tag="m1q", bufs=2)
                        m2q = pa.tile([128, 576], bf16, name="m2q", tag="m2q", bufs=2)
                        m1k = pa.tile([128, 576], bf16, name="m1k", tag="m1k", bufs=2)
                        m2k = pa.tile([128, 576], bf16, name="m2k", tag="m2k", bufs=2)
                        rope_b(
                            ps_b[:, 0:64].rearrange("p (h e) -> p h e", h=HL),
                            m1q, m2q, c3, s3,
                        )
                        rope_a(
                            ps_q, m1q, m2q, qtl,
                            qkb[:, 0:64].rearrange("p (h e) -> p h e", h=HL),
                            c3, s3,
                        )
                        rope_b(
                            ps_b[:, 64:128].rearrange("p (h e) -> p h e", h=HL),
                            m1k, m2k, c3, s3,
                        )
                        rope_a(
                            ps_k, m1k, m2k, ktl,
                            qkb[:, 64:128].rearrange("p (h e) -> p h e", h=HL),
                            c3, s3,
                        )
                        pend = (n, qtl, ktl, qkb)
                    for j in range(9):
                        one_transpose(pend, j)

                # replicate the b-rows to all four 32-row groups (16 data
                # rows each; the other 16 rows were pre-zeroed), in two
                # column halves so the first half's DMAs issue as soon as
                # transposes n<=7 are done (subtile deps)
                for half in range(2):
                    cl, cr = half * 1024, (half + 1) * 1024
                    for hh in range(HL):
                        for j in range(4):
                            nc.sync.dma_start(
                                qTBr[hh][32 * j : 32 * j + 16, cl:cr],
                                qkbT[16 * hh : 16 * hh + 16, cl:cr],
                            )
                            nc.sync.dma_start(
                                kTBr[hh][32 * j : 32 * j + 16, cl:cr],
                                qkbT[64 + 16 * hh : 64 + 16 * hh + 16, cl:cr],
                            )

            # ---------------- Phase B: attention --------------------------
            # v3: 512-wide q items (qb, h, kt).  Per item: one K=32 b-tail
            # (4 consecutive kts' tails issued together on the 4 distinct PE
            # row-groups -> concurrent in the array, ~1/4 the wall cost) +
            # one K=128 main (stop) + exp + 4 PV matmuls.  The exp is split:
            # cols 0:EXS on Scalar (exact ACT Exp), cols EXS:512 on DVE via a
            # Schraudolph bitcast (int16(x*a+b) viewed as bf16, one
            # tensor_scalar op) so neither engine paces the ~660ns/item PE
            # stream.  PSUM: sps 5x1 bank + O1 (ql 0-2) 2x1 + O2 (ql 3 +
            # denom) 1x1 = 8 banks.  O2 is normalized first at head
            # boundaries; the next head's ql3 PV arrives ~760ns later.
            with tc.tile_pool(name="pb", bufs=1) as pb:
                ot = [
                    pb.tile([128, DV], bf16, name=f"o{t}", tag=f"o{t}")
                    for t in range(NT)
                ]
                # phase C's Wo chunks: issue the DMAs now so they stream in
                # during phase B instead of gating the first final matmuls
                wo_tiles = []
                for k in range(5):
                    rows = 128 if k < 4 else 64
                    wot_ = pb.tile([128, D], bf16, name=f"wo{k}", tag=f"wo{k}")
                    nc.sync.dma_start(
                        wot_[0:rows, :], woT[k * 128 : k * 128 + rows, :]
                    )
                    wo_tiles.append(wot_)
                with tc.tile_pool(name="pbps", bufs=1, space="PSUM") as pbps:
                    HD1 = HD + 1
                    QB = 4              # 512-wide q blocks
                    NITEM = QB * HL * NT
                    LOOK = 3            # PV lags scores by 3 items
                    EXS = 256           # scalar-exp columns per 512
                    # Schraudolph bf16 exp: bits = int16(x*EXA + EXB)
                    EXA = SCALE * (2.0 ** 7) / float(np.log(2.0))
                    EXB = 127.0 * 128.0 - 7.5
                    i16 = dt.int16
                    Alu = mybir.AluOpType

                    def decode(idx):
                        qb, rem = divmod(idx, HL * NT)
                        h, kt = divmod(rem, NT)
                        return qb, h, kt

                    sps_pend = {}

                    def stage_tails(s):
                        # 4 consecutive kts' b-tails -> 4 distinct row-groups
                        qb, h, kt0 = decode(s)
                        for j in range(4):
                            kt = kt0 + j
                            rg = kt % 4
                            sps = pbps.tile(
                                [128, 512], f32, name="sps", tag="sc", bufs=6
                            )
                            nc.tensor.matmul(
                                sps[:],
                                kTBr[h][
                                    32 * rg : 32 * rg + 32,
                                    kt * 128 : (kt + 1) * 128,
                                ],
                                qTBr[h][
                                    32 * rg : 32 * rg + 32,
                                    qb * 512 : (qb + 1) * 512,
                                ],
                                start=True,
                                stop=False,
                                tile_position=(32 * rg, 0),
                            )
                            sps_pend[s + j] = sps

                    def stage_main(s):
                        qb, h, kt = decode(s)
                        sps = sps_pend.pop(s)
                        nc.tensor.matmul(
                            sps[:],
                            kTa[h][:, kt * 128 : (kt + 1) * 128],
                            qTa[h][:, qb * 512 : (qb + 1) * 512],
                            start=False,
                            stop=True,
                        )
                        E = pb.tile([128, 512], bf16, name="E", tag="E", bufs=6)
                        nc.scalar.activation(
                            E[:, 0:EXS], sps[:, 0:EXS], AF.Exp, scale=SCALE
                        )
                        nc.vector.tensor_scalar(
                            E[:, EXS:512].bitcast(i16),
                            sps[:, EXS:512],
                            EXA,
                            EXB,
                            Alu.mult,
                            Alu.add,
                        )
                        return E

                    o_ps = None
                    eq = []
                    for s in range(NITEM + LOOK):
                        if s < NITEM:
                            if s % 4 == 0:
                                stage_tails(s)
                            eq.append(stage_main(s))
                        if s < LOOK:
                            continue
                        idx = s - LOOK
                        qb, h, kt = decode(idx)
                        if kt == 0:
                            # (ql0,ql1) and (ql2,ql3) pair up in two banks:
                            # a start=True PV clears its whole bank, so each
                            # new head's PV ql waits on only TWO normalizes
                            # of the previous head, not three
                            O1 = pbps.tile(
                                [128, 2 * HD1], f32, name="O1", tag="O1", bufs=1
                            )
                            O2 = pbps.tile(
                                [128, 2 * HD1], f32, name="O2", tag="O2", bufs=1
                            )
                            o_ps = [
                                O1[:, 0:HD1], O1[:, HD1 : 2 * HD1],
                                O2[:, 0:HD1], O2[:, HD1 : 2 * HD1],
                            ]
                        E = eq.pop(0)
                        for ql in range(4):
                            st = kt == 0 and ql in (0, 2)
                            sp = kt == NT - 1 and ql in (1, 3)
                            nc.tensor.matmul(
                                o_ps[ql][:],
                                E[:, ql * 128 : (ql + 1) * 128],
                                vt[kt][:, HD1 * h : HD1 * (h + 1)],
                                start=st,
                                stop=sp,
                            )
                        if kt == NT - 1:
                            # accumulators are single-buffered: the next
                            # head's bank-clearing PV stalls on BOTH norms of
                            # that bank.  Batch the reciprocals (one [128,2]
                            # op per bank), then O1's norms on DVE and O2's
                            # on Scalar, all at high priority so they beat
                            # the queued exps of the in-flight score stream.
                            with tc.high_priority():
                                dsts = [
                                    ot[4 * qb + ql][:, HD * h : HD * (h + 1)]
                                    for ql in range(4)
                                ]
                                rA = pb.tile([128, 2], f32, name="rA", tag="rA", bufs=2)
                                rB = pb.tile([128, 2], f32, name="rB", tag="rB", bufs=2)
                                nc.vector.reciprocal_approx_fast(
                                    rA.rearrange("p (q o) -> p q o", q=2),
                                    O1.rearrange("p (q e) -> p q e", q=2)[
                                        :, :, HD : HD + 1
                                    ],
                                )
                                nc.vector.reciprocal_approx_fast(
                                    rB.rearrange("p (q o) -> p q o", q=2),
                                    O2.rearrange("p (q e) -> p q e", q=2)[
                                        :, :, HD : HD + 1
                                    ],
                                )
                                nc.scalar.activation(
                                    dsts[2], o_ps[2][:, 0:HD], AF.Copy,
                                    scale=rB[:, 0:1],
                                )
                                nc.scalar.activation(
                                    dsts[3], o_ps[3][:, 0:HD], AF.Copy,
                                    scale=rB[:, 1:2],
                                )
                                nc.vector.tensor_scalar_mul(
                                    dsts[0], o_ps[0][:, 0:HD], rA[:, 0:1]
                                )
                                nc.vector.tensor_scalar_mul(
                                    dsts[1], o_ps[1][:, 0:HD], rA[:, 1:2]
                                )

                # ---------------- Phase C: o^T + final projection ----------
                oTa = [
                    pb.tile([128, T], bf16, name=f"oTa{j}", tag=f"oTa{j}")
                    for j in range(4)
                ]
                oTb = pb.tile([64, T], bf16, name="oTb", tag="oTb")
                with tc.tile_pool(name="pcps", bufs=1, space="PSUM") as pcps:

                    def o_transp(t):
                        for j in range(4):
                            tp = pcps.tile(
                                [128, 128], bf16, name="tpo", tag="otp", bufs=3
                            )
                            nc.tensor.transpose(
                                tp[:],
                                ot[t][:, 128 * j : 128 * (j + 1)],
                                ident_bf[:],
                            )
                            nc.vector.tensor_copy(
                                oTa[j][:, t * 128 : (t + 1) * 128], tp[:]
                            )
                        tpb = pcps.tile([64, 128], bf16, name="tpb", tag="otp", bufs=3)
                        nc.tensor.transpose(
                            tpb[:],
                            ot[t][:, 512:DV],
                            ident_bf[:],
                        )
                        nc.vector.tensor_copy(
                            oTb[:, t * 128 : (t + 1) * 128], tpb[:]
                        )

                    def final(t):
                        fps = [
                            pcps.tile(
                                [128, 384], f32, name=f"fps{j3}", tag=f"f{j3}",
                                bufs=(2 if j3 < 2 else 1),
                            )
                            for j3 in range(3)
                        ]
                        # k-outer / j3-inner: the 3 matmuls of each k share
                        # the o^T stationary, hiding its LDWEIGHTS
                        for k in range(5):
                            rows = 128 if k < 4 else 64
                            lhsT = (
                                oTa[k][:, t * 128 : (t + 1) * 128]
                                if k < 4
                                else oTb[:, t * 128 : (t + 1) * 128]
                            )
                            for j3 in range(3):
                                nc.tensor.matmul(
                                    fps[j3][:],
                                    lhsT,
                                    wo_tiles[k][0:rows, 384 * j3 : 384 * (j3 + 1)],
                                    start=(k == 0),
                                    stop=(k == 4),
                                )
                        for j3 in range(3):
                            fout = pb.tile(
                                [128, 384], f32, name="fout", tag="fout", bufs=10
                            )
                            if (t * 3 + j3) % 2 == 1:
                                nc.vector.tensor_copy(fout[:], fps[j3][:])
                            else:
                                nc.scalar.copy(fout[:], fps[j3][:])
                            nc.sync.dma_start(
                                out[
                                    t * 128 : (t + 1) * 128,
                                    384 * j3 : 384 * (j3 + 1),
                                ],
                                fout[:],
                            )

                    o_transp(0)
                    for t in range(NT):
                        if t + 1 < NT:
                            o_transp(t + 1)
                        final(t)

    nc.compile()
    return nc


def get_nc(debug=False):
    key = bool(debug)
    if key not in _NC_CACHE:
        _NC_CACHE[key] = _build(debug)
    return _NC_CACHE[key]


def make_in_maps(x, cos, sin, Wq, Wk, Wv, Wo):
    import ml_dtypes

    x = np.asarray(x, np.float32)
    cos = np.asarray(cos, np.float32)
    sin = np.asarray(sin, np.float32)
    Wq, Wk, Wv, Wo = (np.asarray(w, np.float32) for w in (Wq, Wk, Wv, Wo))
    cos_bf = cos.astype(ml_dtypes.bfloat16)
    # sign-folded, partner-permuted sin: snP[t,i] = sin[t,(i+72)%144] * s,
    # s = +1 for i<72, -1 for i>=72; makes every rope combine a plain add
    snp = sin[:, (np.arange(HD) + 72) % HD].copy()
    snp[:, 72:] *= -1.0
    snp_bf = snp.astype(ml_dtypes.bfloat16)

    in_maps = []
    for c in range(NCORES):
        b, hg = divmod(c, 2)
        heads = [HL * hg + i for i in range(HL)]

        def w_merged(Wq_, Wk_, Wv_):
            # rows: [q-a 4x128 | k-a 4x128 | q-b 4x16 | k-b 4x16 | v-b 4x16
            #        | v-a 4x128]
            Wsel = np.zeros((WB, D), np.float32)
            for i, g in enumerate(heads):
                a, bb = 144 * g, 144 * g + 128
                Wsel[128 * i : 128 * i + 128] = Wq_[a : a + 128]
                Wsel[512 + 128 * i : 512 + 128 * i + 128] = Wk_[a : a + 128]
                Wsel[1024 + 16 * i : 1024 + 16 * i + 16] = Wq_[bb : bb + 16]
                Wsel[1088 + 16 * i : 1088 + 16 * i + 16] = Wk_[bb : bb + 16]
                Wsel[1152 + 16 * i : 1152 + 16 * i + 16] = Wv_[bb : bb + 16]
                Wsel[1216 + 128 * i : 1216 + 128 * i + 128] = Wv_[a : a + 128]
            return np.ascontiguousarray(Wsel.T)

        wo_sel = np.concatenate([Wo[:, 144 * g : 144 * g + 144] for g in heads], 1)
        in_maps.append(
            {
                "xT": np.ascontiguousarray(x[b].T).astype(ml_dtypes.bfloat16),
                "wbT": w_merged(Wq, Wk, Wv).astype(ml_dtypes.bfloat16),
                "woT": np.ascontiguousarray(wo_sel.T).astype(ml_dtypes.bfloat16),
                "cosN": cos_bf,
                "snPN": snp_bf,
                "identB": np.eye(128, dtype=ml_dtypes.bfloat16),
            }
        )
    return in_maps


def kernel(x, cos, sin, Wq, Wk, Wv, Wo, _trace=False, _trace_kwargs=None):
    from concourse.bass_utils import run_bass_kernel_spmd

    nc = get_nc()
    in_maps = make_in_maps(x, cos, sin, Wq, Wk, Wv, Wo)
    res = run_bass_kernel_spmd(
        nc,
        in_maps,
        list(range(NCORES)),
        trace=_trace,
        **(_trace_kwargs or {}),
    )
    parts = [res.results[c]["out"] for c in range(NCORES)]
    outb = np.stack([parts[2 * b] + parts[2 * b + 1] for b in range(B)])
    if _trace:
        kernel.last_results = res
    return outb.astype(np.float32)



# revision 52
# speedup vs baseline: 1.2467x; 1.1662x over previous
"""Trainium2 Bass kernel for a fused multi-head attention block.

Reference computation (B=4, T=2048, D=1152, H=8, HD=144, full rotary):
    q,k,v = x@Wq.T, x@Wk.T, x@Wv.T   (per head)
    q,k   = rope(q, k, cos, sin)
    o     = softmax(q k^T / sqrt(HD)) v
    out   = o @ Wo.T

Sharding (8 cores): core c = (batch b = c//2, head-group hg = c%2).
Each core computes 4 heads of one batch and a partial output
out_part = o_local @ Wo[:, hg_cols].T ; host sums the two partials per batch.

Per-core structure (v10 — rebuilt from trace analysis of v2, 463us ->
370us on the same box; the binding constraint throughout is PSUM's 8
banks):
  * Phase A (projections+rope+transposes, ~131us): ONE loop; per
    (n t-tile, k d-chunk) FOUR matmuls [q-a 512 | k-a 512 | b' 192 |
    v-a 512] share the x-chunk stationary (b' packs q-b|k-b|v-b 16-dim
    tails of all 4 heads).  rope reads the projection PSUM directly:
    4 DVE muls with a host-precomputed sign-folded/permuted sin table
    (snP) make every combine a plain add, run on GpSimd (otherwise
    idle) to keep DVE from backpressuring the PSUM pool.  v copies out
    on Scalar.  9 PE transposes per n-iteration (4 q, 4 k, 1 shared
    q/k-b block), emitted one-per-k-chunk of the NEXT iteration so the
    single tp PSUM buffer never stalls the PE; copies alternate
    Scalar/DVE.  PSUM: ps_q 2 + ps_k 2 + ps_b 2 + ps_v 1 + tp 1 = 8.
    Warmup matmuls run off a memset tile (no DMA dep) so HAM is warm
    before the first real matmul; the b-row replica tiles (4 row-group
    copies for the score tails) are pre-zeroed on GpSimd and filled by
    16-row DMAs from the transposed qkbT tile.
  * Phase B (attention, ~157us): 512-wide q items (qb, h, kt), flat
    software pipeline, PV lags scores by LOOK=3.  Per item: one K=32
    b-tail + one K=128 main (stop) + exp + 4 PV matmuls (N=145, the
    +1 col = softmax denominator via ones column of v).  The 4 tails
    of each kt-quad are issued together on the 4 distinct PE
    row-groups -> concurrent in the array (~1/4 wall cost); their sps
    allocations (6 bufs) only clear once the exp two items back has
    drained — the exp split is sized so neither engine backs up.
    exp: cols 0:256 exact ACT Exp on Scalar; cols 256:512 on DVE via a
    Schraudolph bitcast exp (one tensor_scalar op: int16(x*a+b) bits
    viewed as bf16, ~1.5% rms rel err on those columns; end-to-end
    rel err 1.2e-2 vs the 2e-2 gate, both engines ~450ns/item).
    Accumulators pack (ql0,ql1)|(ql2,ql3) in two single-buffered
    banks; at head boundaries the batched reciprocal + normalizes run
    at high priority, O1 norms on DVE / O2 norms on Scalar, because
    the next head's bank-clearing PV waits on both norms of its bank.
    PSUM: sps 6 + O1 1 + O2 1 = 8.
  * Phase C (final projection, ~44us): o normalized straight to bf16,
    PE transpose, then k-outer/j3-inner matmuls sharing the o^T
    stationary 1:3.  Wo tiles are DMAed at phase-B start; fout is
    10-deep so the out-DMA (one ~200KB descriptor per [128,384] block)
    never blocks the PSUM->SBUF copies.
  * dtypes: all matmuls bf16 (f32 accum in PSUM); output f32.
"""

import numpy as np

B, T, D, H = 4, 2048, 1152, 8
HL = 4              # heads per core
HD = 144            # head dim
DV = HL * HD        # 576, v/o width
WB = 1728           # merged projection width: q-a 512 | k-a 512 | b' 192 | v-a 512
NT = T // 128       # 16 t-tiles
KC = D // 128       # 9 contraction chunks
SCALE = float(HD) ** -0.5
NCORES = 8

_NC_CACHE = {}


def _build(debug=False):
    import concourse.bacc as bacc
    import concourse.mybir as mybir
    from concourse.tile import TileContext

    dt = mybir.dt
    f32, bf16 = dt.float32, dt.bfloat16
    AF = mybir.ActivationFunctionType

    nc = bacc.Bacc(
        "TRN2",
        target_bir_lowering=False,
        debug=debug,
        enable_asserts=False,
        num_devices=NCORES,
    )

    xT = nc.declare_dram_parameter("xT", [D, T], bf16, isOutput=False)
    wbT = nc.declare_dram_parameter("wbT", [D, WB], bf16, isOutput=False)
    woT = nc.declare_dram_parameter("woT", [DV, D], bf16, isOutput=False)
    cosN = nc.declare_dram_parameter("cosN", [T, HD], bf16, isOutput=False)
    snPN = nc.declare_dram_parameter("snPN", [T, HD], bf16, isOutput=False)
    identB = nc.declare_dram_parameter("identB", [128, 128], bf16, isOutput=False)
    out = nc.declare_dram_parameter("out", [T, D], f32, isOutput=True)

    with TileContext(nc) as tc:
        with tc.tile_pool(name="persist", bufs=1) as P0:
            ident_bf = P0.tile([128, 128], bf16, name="ident_bf", tag="ident_bf")
            nc.sync.dma_start(ident_bf[:], identB[:])

            qTa = [
                P0.tile([128, T], bf16, name=f"qTa{h}", tag=f"qTa{h}")
                for h in range(HL)
            ]
            kTa = [
                P0.tile([128, T], bf16, name=f"kTa{h}", tag=f"kTa{h}")
                for h in range(HL)
            ]
            # b-block rows (transposed): partitions 0:64 = q-b (16 per head,
            # h-major), 64:128 = k-b
            qkbT = P0.tile([128, T], bf16, name="qkbT", tag="qkbT")
            # per-head replicas of the b-block rows at all four 32-row groups,
            # so four score-tail K=32 matmuls can issue to distinct PE
            # row-groups and overlap in the array
            qTBr = [
                P0.tile([128, T], bf16, name=f"qTBr{h}", tag=f"qTBr{h}")
                for h in range(HL)
            ]
            kTBr = [
                P0.tile([128, T], bf16, name=f"kTBr{h}", tag=f"kTBr{h}")
                for h in range(HL)
            ]
            vt = [
                P0.tile([128, HL * (HD + 1)], bf16, name=f"v{t}", tag=f"v{t}")
                for t in range(NT)
            ]

            # ---------------- Phase A: projections + rope + transposes -----
            # v4: ONE loop; per (n,k) FOUR matmuls [q 512 | k 512 | b' 192 |
            # v 512] share the x-chunk stationary (b' = q-b|k-b|v-b packed).
            # PSUM: ps_q 2 + ps_k 2 + ps_b 2 + ps_v 1 + tp 1 = 8 banks.
            # The 9 transposes of iteration n-1 are emitted one-per-k-chunk
            # inside iteration n so the single tp buffer never stalls the PE.
            with tc.tile_pool(name="pa", bufs=1) as pa:
                # pre-zero the replica tiles: the b-row replication below
                # only fills 16 of each 32-row group (the other 16 must be
                # zero for the K=32 tails); GpSimd is idle this early
                # warmup stationary via memset (no DMA dependency) so the
                # HAM-warming dummy matmuls start during runtime preamble;
                # must precede the 8 big replica memsets in the GpSimd queue
                warm_stat = pa.tile([128, 128], bf16, name="warm_stat", tag="warm_stat")
                nc.gpsimd.memset(warm_stat[:], 0.0)
                for hh in range(HL):
                    nc.gpsimd.memset(qTBr[hh][:], 0.0)
                    nc.gpsimd.memset(kTBr[hh][:], 0.0)

                xbig = pa.tile([128, KC * T], bf16, name="xbig", tag="xbig")
                x3 = xbig.rearrange("p (c t) -> p c t", c=KC)
                xs = xT.rearrange("(c p) t -> p c t", p=128)
                xtiles = [x3[:, k] for k in range(KC)]
                cos_sb = pa.tile([128, NT * HD], bf16, name="cos_sb", tag="cos_sb")
                snp_sb = pa.tile([128, NT * HD], bf16, name="snp_sb", tag="snp_sb")
                wbig = pa.tile([128, KC * WB], bf16, name="wbig", tag="wbig")
                wb3 = wbig.rearrange("p (c e) -> p c e", c=KC)
                wb_tiles = [wb3[:, k] for k in range(KC)]
                # first wave at chunk granularity (wb_k + x[k, piece0]
                # interleaved) so matmul (n=0,k) unblocks as pair k lands;
                # later x pieces are single descriptors (Sync-engine issue
                # is ~630 ns per descriptor)
                NP = 8
                PW = T // NP
                wbs = wbT.rearrange("(c p) e -> p c e", p=128)
                # x piece first (smaller, needed with wb chunk cols 0:512
                # for the first matmul); wb chunks split in two descriptors
                # so the q-projection unblocks after ~130KB
                for k in range(KC):
                    nc.sync.dma_start(
                        x3[:, k, 0:PW], xs[:, k, 0:PW]
                    )
                    nc.sync.dma_start(wb3[:, k, 0:512], wbs[:, k, 0:512])
                    nc.sync.dma_start(wb3[:, k, 512:WB], wbs[:, k, 512:WB])
                # pieces 1-3 split across two DMA engines: a single 576KB
                # descriptor delivers at ~25GB/s, marginal vs the PE's
                # consumption rate (the early ~780ns/n starvation gaps)
                for p in range(1, NP):
                    if p <= 3:
                        nc.sync.dma_start(
                            x3[:, 0:5, p * PW : (p + 1) * PW],
                            xs[:, 0:5, p * PW : (p + 1) * PW],
                        )
                        nc.sync.dma_start(
                            x3[:, 5:KC, p * PW : (p + 1) * PW],
                            xs[:, 5:KC, p * PW : (p + 1) * PW],
                        )
                    else:
                        nc.sync.dma_start(
                            x3[:, :, p * PW : (p + 1) * PW],
                            xs[:, :, p * PW : (p + 1) * PW],
                        )
                nc.sync.dma_start(
                    cos_sb.rearrange("p (n r) -> p n r", n=NT),
                    cosN.rearrange("(n p) r -> p n r", p=128),
                )
                nc.sync.dma_start(
                    snp_sb.rearrange("p (n r) -> p n r", n=NT),
                    snPN.rearrange("(n p) r -> p n r", p=128),
                )

                def trig3(sb, n):
                    # [128, 144] row block for t-tile n, broadcast over 4 heads
                    return (
                        sb[:, n * HD : (n + 1) * HD]
                        .rearrange("p (o r) -> p o r", o=1)
                        .to_broadcast([128, HL, HD])
                    )

                def rope_b(ps_bq, m1, m2, cos3, snp3):
                    """The two b-dim muls -- issued for q AND k before
                    anything else so ps_b (bufs=1) frees early."""
                    m1b = m1[:, 512:576].rearrange("p (h e) -> p h e", h=HL)
                    m2b = m2[:, 512:576].rearrange("p (h e) -> p h e", h=HL)
                    nc.vector.tensor_mul(m1b[:], ps_bq[:], cos3[:, :, 128:144])
                    nc.vector.tensor_mul(m2b[:], ps_bq[:], snp3[:, :, 128:144])

                def rope_a(ps_a, m1, m2, qtl, ob3, cos3, snp3):
                    """ps_a [128,512] f32 (4 a-blocks) -> qtl [128,512] bf16
                    (a) and ob3 [128,4,16] bf16 (b) with rotary applied.

                    m1[j] = q[j]*cos[j]; m2[j] = q[j]*snP[j] where
                    snP[i] = sin[(i+72)%144] * (+1 if i<72 else -1), so every
                    combine is a plain add: out[j] = m1[j] + m2[(j+72)%144].
                    """
                    pa3 = ps_a.rearrange("p (h e) -> p h e", h=HL)
                    m1a = m1[:, 0:512].rearrange("p (h e) -> p h e", h=HL)
                    m1b = m1[:, 512:576].rearrange("p (h e) -> p h e", h=HL)
                    m2a = m2[:, 0:512].rearrange("p (h e) -> p h e", h=HL)
                    m2b = m2[:, 512:576].rearrange("p (h e) -> p h e", h=HL)
                    v = nc.vector
                    v.tensor_mul(m1a[:], pa3[:], cos3[:, :, 0:128])
                    v.tensor_mul(m2a[:], pa3[:], snp3[:, :, 0:128])
                    oa = qtl.rearrange("p (h e) -> p h e", h=HL)
                    # all-bf16 SBUF operands; run the combines on GpSimd
                    # (idle otherwise) so DVE only does the 4 PSUM-read muls
                    g = nc.gpsimd
                    g.tensor_add(oa[:, :, 0:56], m1a[:, :, 0:56], m2a[:, :, 72:128])
                    g.tensor_add(oa[:, :, 56:72], m1a[:, :, 56:72], m2b[:, :, 0:16])
                    g.tensor_add(oa[:, :, 72:128], m1a[:, :, 72:128], m2a[:, :, 0:56])
                    g.tensor_add(ob3[:], m1b[:], m2a[:, :, 56:72])

                with tc.tile_pool(name="paqps", bufs=1, space="PSUM") as paqps:
                    warm_ps = paqps.tile(
                        [128, 512], f32, name="warm_ps", tag="pv", bufs=1
                    )
                    with tc.high_priority():
                        for _ in range(10):
                            nc.tensor.matmul(
                                warm_ps[:, 0:128], warm_stat[:], warm_stat[:],
                                start=True, stop=True,
                            )

                    def one_transpose(pend, j):
                        n, qtl, ktl, qkb = pend
                        tp = paqps.tile(
                            [128, 128], bf16, name="tp", tag="tp", bufs=1
                        )
                        if j < 4:
                            src, dst = qtl[:, 128 * j : 128 * (j + 1)], qTa[j]
                        elif j < 8:
                            src, dst = ktl[:, 128 * (j - 4) : 128 * (j - 3)], kTa[j - 4]
                        else:
                            src, dst = qkb[:], qkbT
                        nc.tensor.transpose(tp[:], src, ident_bf[:])
                        if j % 2:
                            nc.scalar.copy(dst[:, n * 128 : (n + 1) * 128], tp[:])
                        else:
                            nc.vector.tensor_copy(
                                dst[:, n * 128 : (n + 1) * 128], tp[:]
                            )

                    pend = None
                    for n in range(NT):
                        ps_q = paqps.tile(
                            [128, 512], f32, name="ps_q", tag="psq", bufs=2
                        )
                        ps_k = paqps.tile(
                            [128, 512], f32, name="ps_k", tag="psk", bufs=2
                        )
                        ps_b = paqps.tile(
                            [128, 192], f32, name="ps_b", tag="psb", bufs=2
                        )
                        ps_v = paqps.tile(
                            [128, 512], f32, name="ps_v", tag="pv", bufs=1
                        )
                        for k in range(KC):
                            st, sp = k == 0, k == KC - 1
                            lhs = xtiles[k][:, n * 128 : (n + 1) * 128]
                            nc.tensor.matmul(
                                ps_q[:], lhs, wb_tiles[k][:, 0:512],
                                start=st, stop=sp,
                            )
                            nc.tensor.matmul(
                                ps_k[:], lhs, wb_tiles[k][:, 512:1024],
                                start=st, stop=sp,
                            )
                            nc.tensor.matmul(
                                ps_b[:], lhs, wb_tiles[k][:, 1024:1216],
                                start=st, stop=sp,
                            )
                            nc.tensor.matmul(
                                ps_v[:], lhs, wb_tiles[k][:, 1216:WB],
                                start=st, stop=sp,
                            )
                            if pend is not None:
                                one_transpose(pend, k)
                        v3 = vt[n].rearrange("p (h e) -> p h e", h=HL)
                        c3, s3 = trig3(cos_sb, n), trig3(snp_sb, n)
                        # scalar engine: DVE is saturated by rope in the
                        # steady state and was backpressuring the v matmuls
                        nc.scalar.copy(
                            v3[:, :, 128:HD],
                            ps_b[:, 128:192].rearrange("p (h e) -> p h e", h=HL),
                        )
                        nc.scalar.copy(
                            v3[:, :, 0:128],
                            ps_v.rearrange("p (h e) -> p h e", h=HL),
                        )
                        nc.gpsimd.memset(v3[:, :, HD : HD + 1], 1.0)
                        qtl = pa.tile([128, 512], bf16, name="qtl", tag="qtl", bufs=2)
                        ktl = pa.tile([128, 512], bf16, name="ktl", tag="ktl", bufs=2)
                        qkb = pa.tile([128, 128], bf16, name="qkb", tag="qkb", bufs=2)
                        m1q = pa.tile([128, 576], bf16, name="m1q", tag="m1q", bufs=2)
                        m2q = pa.tile([128, 576], bf16, name="m2q", tag="m2q", bufs=2)
                        m1k = pa.tile([128, 576], bf16, name="m1k", tag="m1k", bufs=2)
                        m2k = pa.tile([128, 576], bf16, name="m2k", tag="m2k", bufs=2)
                        rope_b(
                            ps_b[:, 0:64].rearrange("p (h e) -> p h e", h=HL),
                            m1q, m2q, c3, s3,
                        )
                        rope_a(
                            ps_q, m1q, m2q, qtl,
                            qkb[:, 0:64].rearrange("p (h e) -> p h e", h=HL),
                            c3, s3,
                        )
                        rope_b(
                            ps_b[:, 64:128].rearrange("p (h e) -> p h e", h=HL),
                            m1k, m2k, c3, s3,
                        )
                        rope_a(
                            ps_k, m1k, m2k, ktl,
                            qkb[:, 64:128].rearrange("p (h e) -> p h e", h=HL),
                            c3, s3,
                        )
                        pend = (n, qtl, ktl, qkb)
                    for j in range(9):
                        one_transpose(pend, j)

                # replicate the b-rows to all four 32-row groups (16 data
                # rows each; the other 16 rows were pre-zeroed), in two
                # column halves so the first half's DMAs issue as soon as
                # transposes n<=7 are done (subtile deps)
                for half in range(2):
                    cl, cr = half * 1024, (half + 1) * 1024
                    for hh in range(HL):
                        for j in range(4):
                            nc.sync.dma_start(
                                qTBr[hh][32 * j : 32 * j + 16, cl:cr],
                                qkbT[16 * hh : 16 * hh + 16, cl:cr],
                            )
                            nc.sync.dma_start(
                                kTBr[hh][32 * j : 32 * j + 16, cl:cr],
                                qkbT[64 + 16 * hh : 64 + 16 * hh + 16, cl:cr],
                            )

            # ---------------- Phase B: attention --------------------------
            # v3: 512-wide q items (qb, h, kt).  Per item: one K=32 b-tail
            # (4 consecutive kts' tails issued together on the 4 distinct PE
            # row-groups -> concurrent in the array, ~1/4 the wall cost) +
            # one K=128 main (stop) + exp + 4 PV matmuls.  The exp is split:
            # cols 0:EXS on Scalar (exact ACT Exp), cols EXS:512 on DVE via a
            # Schraudolph bitcast (int16(x*a+b) viewed as bf16, one
            # tensor_scalar op) so neither engine paces the ~660ns/item PE
            # stream.  PSUM: sps 5x1 bank + O1 (ql 0-2) 2x1 + O2 (ql 3 +
            # denom) 1x1 = 8 banks.  O2 is normalized first at head
            # boundaries; the next head's ql3 PV arrives ~760ns later.
            with tc.tile_pool(name="pb", bufs=1) as pb:
                ot = [
                    pb.tile([128, DV], bf16, name=f"o{t}", tag=f"o{t}")
                    for t in range(NT)
                ]
                # phase C's Wo chunks: issue the DMAs now so they stream in
                # during phase B instead of gating the first final matmuls
                wo_tiles = []
                for k in range(5):
                    rows = 128 if k < 4 else 64
                    wot_ = pb.tile([128, D], bf16, name=f"wo{k}", tag=f"wo{k}")
                    nc.sync.dma_start(
                        wot_[0:rows, :], woT[k * 128 : k * 128 + rows, :]
                    )
                    wo_tiles.append(wot_)
                with tc.tile_pool(name="pbps", bufs=1, space="PSUM") as pbps:
                    HD1 = HD + 1
                    QB = 4              # 512-wide q blocks
                    NITEM = QB * HL * NT
                    LOOK = 3            # PV lags scores by 3 items
                    EXS = 256           # scalar-exp columns per 512
                    # Schraudolph bf16 exp: bits = int16(x*EXA + EXB)
                    EXA = SCALE * (2.0 ** 7) / float(np.log(2.0))
                    EXB = 127.0 * 128.0 - 7.5
                    i16 = dt.int16
                    Alu = mybir.AluOpType

                    def decode(idx):
                        qb, rem = divmod(idx, HL * NT)
                        h, kt = divmod(rem, NT)
                        return qb, h, kt

                    sps_pend = {}

                    def stage_tails(s):
                        # 4 consecutive kts' b-tails -> 4 distinct row-groups
                        qb, h, kt0 = decode(s)
                        for j in range(4):
                            kt = kt0 + j
                            rg = kt % 4
                            sps = pbps.tile(
                                [128, 512], f32, name="sps", tag="sc", bufs=6
                            )
                            nc.tensor.matmul(
                                sps[:],
                                kTBr[h][
                                    32 * rg : 32 * rg + 32,
                                    kt * 128 : (kt + 1) * 128,
                                ],
                                qTBr[h][
                                    32 * rg : 32 * rg + 32,
                                    qb * 512 : (qb + 1) * 512,
                                ],
                                start=True,
                                stop=False,
                                tile_position=(32 * rg, 0),
                            )
                            sps_pend[s + j] = sps

                    def stage_main(s):
                        qb, h, kt = decode(s)
                        sps = sps_pend.pop(s)
                        nc.tensor.matmul(
                            sps[:],
                            kTa[h][:, kt * 128 : (kt + 1) * 128],
                            qTa[h][:, qb * 512 : (qb + 1) * 512],
                            start=False,
                            stop=True,
                        )
                        E = pb.tile([128, 512], bf16, name="E", tag="E", bufs=6)
                        nc.scalar.activation(
                            E[:, 0:EXS], sps[:, 0:EXS], AF.Exp, scale=SCALE
                        )
                        nc.vector.tensor_scalar(
                            E[:, EXS:512].bitcast(i16),
                            sps[:, EXS:512],
                            EXA,
                            EXB,
                            Alu.mult,
                            Alu.add,
                        )
                        return E

                    box = {}
                    eq = []

                    def pv_block(idx):
                        qb, h, kt = decode(idx)
                        if kt == 0:
                            # (ql0,ql1) and (ql2,ql3) pair up in two banks:
                            # a start=True PV clears its whole bank, so each
                            # new head's PV ql waits on only TWO normalizes
                            # of the previous head, not three
                            box["O1"] = pbps.tile(
                                [128, 2 * HD1], f32, name="O1", tag="O1", bufs=1
                            )
                            box["O2"] = pbps.tile(
                                [128, 2 * HD1], f32, name="O2", tag="O2", bufs=1
                            )
                        O1, O2 = box["O1"], box["O2"]
                        o_ps = [
                            O1[:, 0:HD1], O1[:, HD1 : 2 * HD1],
                            O2[:, 0:HD1], O2[:, HD1 : 2 * HD1],
                        ]
                        E = eq.pop(0)
                        for ql in range(4):
                            st = kt == 0 and ql in (0, 2)
                            sp = kt == NT - 1 and ql in (1, 3)
                            nc.tensor.matmul(
                                o_ps[ql][:],
                                E[:, ql * 128 : (ql + 1) * 128],
                                vt[kt][:, HD1 * h : HD1 * (h + 1)],
                                start=st,
                                stop=sp,
                            )
                        if kt == NT - 1:
                            # accumulators are single-buffered: the next
                            # head's bank-clearing PV stalls on BOTH norms of
                            # that bank.  Batch the reciprocals (one [128,2]
                            # op per bank), then O1's norms on DVE and O2's
                            # on Scalar, all at high priority so they beat
                            # the queued exps of the in-flight score stream.
                            with tc.high_priority():
                                dsts = [
                                    ot[4 * qb + ql][:, HD * h : HD * (h + 1)]
                                    for ql in range(4)
                                ]
                                rA = pb.tile([128, 2], f32, name="rA", tag="rA", bufs=2)
                                rB = pb.tile([128, 2], f32, name="rB", tag="rB", bufs=2)
                                nc.vector.reciprocal_approx_fast(
                                    rA.rearrange("p (q o) -> p q o", q=2),
                                    O1.rearrange("p (q e) -> p q e", q=2)[
                                        :, :, HD : HD + 1
                                    ],
                                )
                                nc.vector.reciprocal_approx_fast(
                                    rB.rearrange("p (q o) -> p q o", q=2),
                                    O2.rearrange("p (q e) -> p q e", q=2)[
                                        :, :, HD : HD + 1
                                    ],
                                )
                                nc.scalar.activation(
                                    dsts[2], o_ps[2][:, 0:HD], AF.Copy,
                                    scale=rB[:, 0:1],
                                )
                                nc.scalar.activation(
                                    dsts[3], o_ps[3][:, 0:HD], AF.Copy,
                                    scale=rB[:, 1:2],
                                )
                                nc.vector.tensor_scalar_mul(
                                    dsts[0], o_ps[0][:, 0:HD], rA[:, 0:1]
                                )
                                nc.vector.tensor_scalar_mul(
                                    dsts[1], o_ps[1][:, 0:HD], rA[:, 1:2]
                                )

                    for s in range(NITEM + LOOK):
                        if s < NITEM and s % 4 == 0:
                            # quad start: PVs BEFORE the main so the main's
                            # LDWEIGHTS hides behind them instead of sitting
                            # exposed right after the 4 concurrent tails
                            stage_tails(s)
                            if s >= LOOK:
                                pv_block(s - LOOK)
                            eq.append(stage_main(s))
                        else:
                            if s < NITEM:
                                eq.append(stage_main(s))
                            if s >= LOOK:
                                pv_block(s - LOOK)

                # ---------------- Phase C: o^T + final projection ----------
                oTa = [
                    pb.tile([128, T], bf16, name=f"oTa{j}", tag=f"oTa{j}")
                    for j in range(4)
                ]
                oTb = pb.tile([64, T], bf16, name="oTb", tag="oTb")
                with tc.tile_pool(name="pcps", bufs=1, space="PSUM") as pcps:

                    def o_transp(t):
                        for j in range(4):
                            tp = pcps.tile(
                                [128, 128], bf16, name="tpo", tag="otp", bufs=3
                            )
                            nc.tensor.transpose(
                                tp[:],
                                ot[t][:, 128 * j : 128 * (j + 1)],
                                ident_bf[:],
                            )
                            nc.vector.tensor_copy(
                                oTa[j][:, t * 128 : (t + 1) * 128], tp[:]
                            )
                        tpb = pcps.tile([64, 128], bf16, name="tpb", tag="otp", bufs=3)
                        nc.tensor.transpose(
                            tpb[:],
                            ot[t][:, 512:DV],
                            ident_bf[:],
                        )
                        nc.vector.tensor_copy(
                            oTb[:, t * 128 : (t + 1) * 128], tpb[:]
                        )

                    def final(t):
                        fps = [
                            pcps.tile(
                                [128, 384], f32, name=f"fps{j3}", tag=f"f{j3}",
                                bufs=(2 if j3 < 2 else 1),
                            )
                            for j3 in range(3)
                        ]
                        # k-outer / j3-inner: the 3 matmuls of each k share
                        # the o^T stationary, hiding its LDWEIGHTS
                        for k in range(5):
                            rows = 128 if k < 4 else 64
                            lhsT = (
                                oTa[k][:, t * 128 : (t + 1) * 128]
                                if k < 4
                                else oTb[:, t * 128 : (t + 1) * 128]
                            )
                            for j3 in range(3):
                                nc.tensor.matmul(
                                    fps[j3][:],
                                    lhsT,
                                    wo_tiles[k][0:rows, 384 * j3 : 384 * (j3 + 1)],
                                    start=(k == 0),
                                    stop=(k == 4),
                                )
                        for j3 in range(3):
                            fout = pb.tile(
                                [128, 384], f32, name="fout", tag="fout", bufs=10
                            )
                            if (t * 3 + j3) % 2 == 1:
                                nc.vector.tensor_copy(fout[:], fps[j3][:])
                            else:
                                nc.scalar.copy(fout[:], fps[j3][:])
                            nc.sync.dma_start(
                                out[
                                    t * 128 : (t + 1) * 128,
                                    384 * j3 : 384 * (j3 + 1),
                                ],
                                fout[:],
                            )

                    o_transp(0)
                    for t in range(NT):
                        if t + 1 < NT:
                            o_transp(t + 1)
                        final(t)

    nc.compile()
    return nc


def get_nc(debug=False):
    key = bool(debug)
    if key not in _NC_CACHE:
        _NC_CACHE[key] = _build(debug)
    return _NC_CACHE[key]


def make_in_maps(x, cos, sin, Wq, Wk, Wv, Wo):
    import ml_dtypes

    x = np.asarray(x, np.float32)
    cos = np.asarray(cos, np.float32)
    sin = np.asarray(sin, np.float32)
    Wq, Wk, Wv, Wo = (np.asarray(w, np.float32) for w in (Wq, Wk, Wv, Wo))
    cos_bf = cos.astype(ml_dtypes.bfloat16)
    # sign-folded, partner-permuted sin: snP[t,i] = sin[t,(i+72)%144] * s,
    # s = +1 for i<72, -1 for i>=72; makes every rope combine a plain add
    snp = sin[:, (np.arange(HD) + 72) % HD].copy()
    snp[:, 72:] *= -1.0
    snp_bf = snp.astype(ml_dtypes.bfloat16)

    in_maps = []
    for c in range(NCORES):
        b, hg = divmod(c, 2)
        heads = [HL * hg + i for i in range(HL)]

        def w_merged(Wq_, Wk_, Wv_):
            # rows: [q-a 4x128 | k-a 4x128 | q-b 4x16 | k-b 4x16 | v-b 4x16
            #        | v-a 4x128]
            Wsel = np.zeros((WB, D), np.float32)
            for i, g in enumerate(heads):
                a, bb = 144 * g, 144 * g + 128
                Wsel[128 * i : 128 * i + 128] = Wq_[a : a + 128]
                Wsel[512 + 128 * i : 512 + 128 * i + 128] = Wk_[a : a + 128]
                Wsel[1024 + 16 * i : 1024 + 16 * i + 16] = Wq_[bb : bb + 16]
                Wsel[1088 + 16 * i : 1088 + 16 * i + 16] = Wk_[bb : bb + 16]
                Wsel[1152 + 16 * i : 1152 + 16 * i + 16] = Wv_[bb : bb + 16]
                Wsel[1216 + 128 * i : 1216 + 128 * i + 128] = Wv_[a : a + 128]
            return np.ascontiguousarray(Wsel.T)

        wo_sel = np.concatenate([Wo[:, 144 * g : 144 * g + 144] for g in heads], 1)
        in_maps.append(
            {
                "xT": np.ascontiguousarray(x[b].T).astype(ml_dtypes.bfloat16),
                "wbT": w_merged(Wq, Wk, Wv).astype(ml_dtypes.bfloat16),
                "woT": np.ascontiguousarray(wo_sel.T).astype(ml_dtypes.bfloat16),
                "cosN": cos_bf,
                "snPN": snp_bf,
                "identB": np.eye(128, dtype=ml_dtypes.bfloat16),
            }
        )
    return in_maps


def kernel(x, cos, sin, Wq, Wk, Wv, Wo, _trace=False, _trace_kwargs=None):
    from concourse.bass_utils import run_bass_kernel_spmd

    nc = get_nc()
    in_maps = make_in_maps(x, cos, sin, Wq, Wk, Wv, Wo)
    res = run_bass_kernel_spmd(
        nc,
        in_maps,
        list(range(NCORES)),
        trace=_trace,
        **(_trace_kwargs or {}),
    )
    parts = [res.results[c]["out"] for c in range(NCORES)]
    outb = np.stack([parts[2 * b] + parts[2 * b + 1] for b in range(B)])
    if _trace:
        kernel.last_results = res
    return outb.astype(np.float32)



# revision 53
# speedup vs baseline: 1.2523x; 1.0045x over previous
"""Trainium2 Bass kernel for a fused multi-head attention block.

Reference computation (B=4, T=2048, D=1152, H=8, HD=144, full rotary):
    q,k,v = x@Wq.T, x@Wk.T, x@Wv.T   (per head)
    q,k   = rope(q, k, cos, sin)
    o     = softmax(q k^T / sqrt(HD)) v
    out   = o @ Wo.T

Sharding (8 cores): core c = (batch b = c//2, head-group hg = c%2).
Each core computes 4 heads of one batch and a partial output
out_part = o_local @ Wo[:, hg_cols].T ; host sums the two partials per batch.

Per-core structure (v10 — rebuilt from trace analysis of v2, 463us ->
370us on the same box; the binding constraint throughout is PSUM's 8
banks):
  * Phase A (projections+rope+transposes, ~131us): ONE loop; per
    (n t-tile, k d-chunk) FOUR matmuls [q-a 512 | k-a 512 | b' 192 |
    v-a 512] share the x-chunk stationary (b' packs q-b|k-b|v-b 16-dim
    tails of all 4 heads).  rope reads the projection PSUM directly:
    4 DVE muls with a host-precomputed sign-folded/permuted sin table
    (snP) make every combine a plain add, run on GpSimd (otherwise
    idle) to keep DVE from backpressuring the PSUM pool.  v copies out
    on Scalar.  9 PE transposes per n-iteration (4 q, 4 k, 1 shared
    q/k-b block), emitted one-per-k-chunk of the NEXT iteration so the
    single tp PSUM buffer never stalls the PE; copies alternate
    Scalar/DVE.  PSUM: ps_q 2 + ps_k 2 + ps_b 2 + ps_v 1 + tp 1 = 8.
    Warmup matmuls run off a memset tile (no DMA dep) so HAM is warm
    before the first real matmul; the b-row replica tiles (4 row-group
    copies for the score tails) are pre-zeroed on GpSimd and filled by
    16-row DMAs from the transposed qkbT tile.
  * Phase B (attention, ~157us): 512-wide q items (qb, h, kt), flat
    software pipeline, PV lags scores by LOOK=3.  Per item: one K=32
    b-tail + one K=128 main (stop) + exp + 4 PV matmuls (N=145, the
    +1 col = softmax denominator via ones column of v).  The 4 tails
    of each kt-quad are issued together on the 4 distinct PE
    row-groups -> concurrent in the array (~1/4 wall cost); their sps
    allocations (6 bufs) only clear once the exp two items back has
    drained — the exp split is sized so neither engine backs up.
    exp: cols 0:256 exact ACT Exp on Scalar; cols 256:512 on DVE via a
    Schraudolph bitcast exp (one tensor_scalar op: int16(x*a+b) bits
    viewed as bf16, ~1.5% rms rel err on those columns; end-to-end
    rel err 1.2e-2 vs the 2e-2 gate, both engines ~450ns/item).
    Accumulators pack (ql0,ql1)|(ql2,ql3) in two single-buffered
    banks; at head boundaries the batched reciprocal + normalizes run
    at high priority, O1 norms on DVE / O2 norms on Scalar, because
    the next head's bank-clearing PV waits on both norms of its bank.
    PSUM: sps 6 + O1 1 + O2 1 = 8.
  * Phase C (final projection, ~44us): o normalized straight to bf16,
    PE transpose, then k-outer/j3-inner matmuls sharing the o^T
    stationary 1:3.  Wo tiles are DMAed at phase-B start; fout is
    10-deep so the out-DMA (one ~200KB descriptor per [128,384] block)
    never blocks the PSUM->SBUF copies.
  * dtypes: all matmuls bf16 (f32 accum in PSUM); output f32.
"""

import numpy as np

B, T, D, H = 4, 2048, 1152, 8
HL = 4              # heads per core
HD = 144            # head dim
DV = HL * HD        # 576, v/o width
WB = 1728           # merged projection width: q-a 512 | k-a 512 | b' 192 | v-a 512
NT = T // 128       # 16 t-tiles
KC = D // 128       # 9 contraction chunks
SCALE = float(HD) ** -0.5
NCORES = 8

_NC_CACHE = {}


def _build(debug=False):
    import concourse.bacc as bacc
    import concourse.mybir as mybir
    from concourse.tile import TileContext

    dt = mybir.dt
    f32, bf16 = dt.float32, dt.bfloat16
    AF = mybir.ActivationFunctionType

    nc = bacc.Bacc(
        "TRN2",
        target_bir_lowering=False,
        debug=debug,
        enable_asserts=False,
        num_devices=NCORES,
    )

    xT = nc.declare_dram_parameter("xT", [D, T], bf16, isOutput=False)
    wbT = nc.declare_dram_parameter("wbT", [D, WB], bf16, isOutput=False)
    woT = nc.declare_dram_parameter("woT", [DV, D], bf16, isOutput=False)
    cosN = nc.declare_dram_parameter("cosN", [T, HD], bf16, isOutput=False)
    snPN = nc.declare_dram_parameter("snPN", [T, HD], bf16, isOutput=False)
    identB = nc.declare_dram_parameter("identB", [128, 128], bf16, isOutput=False)
    out = nc.declare_dram_parameter("out", [T, D], f32, isOutput=True)

    with TileContext(nc) as tc:
        with tc.tile_pool(name="persist", bufs=1) as P0:
            ident_bf = P0.tile([128, 128], bf16, name="ident_bf", tag="ident_bf")
            nc.sync.dma_start(ident_bf[:], identB[:])

            qTa = [
                P0.tile([128, T], bf16, name=f"qTa{h}", tag=f"qTa{h}")
                for h in range(HL)
            ]
            kTa = [
                P0.tile([128, T], bf16, name=f"kTa{h}", tag=f"kTa{h}")
                for h in range(HL)
            ]
            # b-block rows (transposed): partitions 0:64 = q-b (16 per head,
            # h-major), 64:128 = k-b
            qkbT = P0.tile([128, T], bf16, name="qkbT", tag="qkbT")
            # per-head replicas of the b-block rows at all four 32-row groups,
            # so four score-tail K=32 matmuls can issue to distinct PE
            # row-groups and overlap in the array
            qTBr = [
                P0.tile([128, T], bf16, name=f"qTBr{h}", tag=f"qTBr{h}")
                for h in range(HL)
            ]
            kTBr = [
                P0.tile([128, T], bf16, name=f"kTBr{h}", tag=f"kTBr{h}")
                for h in range(HL)
            ]
            vt = [
                P0.tile([128, HL * (HD + 1)], bf16, name=f"v{t}", tag=f"v{t}")
                for t in range(NT)
            ]

            # ---------------- Phase A: projections + rope + transposes -----
            # v4: ONE loop; per (n,k) FOUR matmuls [q 512 | k 512 | b' 192 |
            # v 512] share the x-chunk stationary (b' = q-b|k-b|v-b packed).
            # PSUM: ps_q 2 + ps_k 2 + ps_b 2 + ps_v 1 + tp 1 = 8 banks.
            # The 9 transposes of iteration n-1 are emitted one-per-k-chunk
            # inside iteration n so the single tp buffer never stalls the PE.
            with tc.tile_pool(name="pa", bufs=1) as pa:
                # pre-zero the replica tiles: the b-row replication below
                # only fills 16 of each 32-row group (the other 16 must be
                # zero for the K=32 tails); GpSimd is idle this early
                # warmup stationary via memset (no DMA dependency) so the
                # HAM-warming dummy matmuls start during runtime preamble;
                # must precede the 8 big replica memsets in the GpSimd queue
                warm_stat = pa.tile([128, 128], bf16, name="warm_stat", tag="warm_stat")
                nc.gpsimd.memset(warm_stat[:], 0.0)
                for hh in range(HL):
                    nc.gpsimd.memset(qTBr[hh][:], 0.0)
                    nc.gpsimd.memset(kTBr[hh][:], 0.0)

                xbig = pa.tile([128, KC * T], bf16, name="xbig", tag="xbig")
                x3 = xbig.rearrange("p (c t) -> p c t", c=KC)
                xs = xT.rearrange("(c p) t -> p c t", p=128)
                xtiles = [x3[:, k] for k in range(KC)]
                cos_sb = pa.tile([128, NT * HD], bf16, name="cos_sb", tag="cos_sb")
                snp_sb = pa.tile([128, NT * HD], bf16, name="snp_sb", tag="snp_sb")
                wbig = pa.tile([128, KC * WB], bf16, name="wbig", tag="wbig")
                wb3 = wbig.rearrange("p (c e) -> p c e", c=KC)
                wb_tiles = [wb3[:, k] for k in range(KC)]
                # first wave at chunk granularity (wb_k + x[k, piece0]
                # interleaved) so matmul (n=0,k) unblocks as pair k lands;
                # later x pieces are single descriptors (Sync-engine issue
                # is ~630 ns per descriptor)
                NP = 8
                PW = T // NP
                wbs = wbT.rearrange("(c p) e -> p c e", p=128)
                # x piece first (smaller, needed with wb chunk cols 0:512
                # for the first matmul); wb chunks split in two descriptors
                # so the q-projection unblocks after ~130KB
                for k in range(KC):
                    nc.sync.dma_start(
                        x3[:, k, 0:PW], xs[:, k, 0:PW]
                    )
                    nc.sync.dma_start(wb3[:, k, 0:512], wbs[:, k, 0:512])
                    nc.sync.dma_start(wb3[:, k, 512:WB], wbs[:, k, 512:WB])
                for p in range(1, NP):
                    nc.sync.dma_start(
                        x3[:, :, p * PW : (p + 1) * PW],
                        xs[:, :, p * PW : (p + 1) * PW],
                    )
                nc.sync.dma_start(
                    cos_sb.rearrange("p (n r) -> p n r", n=NT),
                    cosN.rearrange("(n p) r -> p n r", p=128),
                )
                nc.sync.dma_start(
                    snp_sb.rearrange("p (n r) -> p n r", n=NT),
                    snPN.rearrange("(n p) r -> p n r", p=128),
                )

                def trig3(sb, n):
                    # [128, 144] row block for t-tile n, broadcast over 4 heads
                    return (
                        sb[:, n * HD : (n + 1) * HD]
                        .rearrange("p (o r) -> p o r", o=1)
                        .to_broadcast([128, HL, HD])
                    )

                def rope_b(ps_bq, m1, m2, cos3, snp3):
                    """The two b-dim muls -- issued for q AND k before
                    anything else so ps_b (bufs=1) frees early."""
                    m1b = m1[:, 512:576].rearrange("p (h e) -> p h e", h=HL)
                    m2b = m2[:, 512:576].rearrange("p (h e) -> p h e", h=HL)
                    nc.vector.tensor_mul(m1b[:], ps_bq[:], cos3[:, :, 128:144])
                    nc.vector.tensor_mul(m2b[:], ps_bq[:], snp3[:, :, 128:144])

                def rope_a(ps_a, m1, m2, qtl, ob3, cos3, snp3):
                    """ps_a [128,512] f32 (4 a-blocks) -> qtl [128,512] bf16
                    (a) and ob3 [128,4,16] bf16 (b) with rotary applied.

                    m1[j] = q[j]*cos[j]; m2[j] = q[j]*snP[j] where
                    snP[i] = sin[(i+72)%144] * (+1 if i<72 else -1), so every
                    combine is a plain add: out[j] = m1[j] + m2[(j+72)%144].
                    """
                    pa3 = ps_a.rearrange("p (h e) -> p h e", h=HL)
                    m1a = m1[:, 0:512].rearrange("p (h e) -> p h e", h=HL)
                    m1b = m1[:, 512:576].rearrange("p (h e) -> p h e", h=HL)
                    m2a = m2[:, 0:512].rearrange("p (h e) -> p h e", h=HL)
                    m2b = m2[:, 512:576].rearrange("p (h e) -> p h e", h=HL)
                    v = nc.vector
                    v.tensor_mul(m1a[:], pa3[:], cos3[:, :, 0:128])
                    v.tensor_mul(m2a[:], pa3[:], snp3[:, :, 0:128])
                    oa = qtl.rearrange("p (h e) -> p h e", h=HL)
                    # all-bf16 SBUF operands; run the combines on GpSimd
                    # (idle otherwise) so DVE only does the 4 PSUM-read muls
                    g = nc.gpsimd
                    g.tensor_add(oa[:, :, 0:56], m1a[:, :, 0:56], m2a[:, :, 72:128])
                    g.tensor_add(oa[:, :, 56:72], m1a[:, :, 56:72], m2b[:, :, 0:16])
                    g.tensor_add(oa[:, :, 72:128], m1a[:, :, 72:128], m2a[:, :, 0:56])
                    g.tensor_add(ob3[:], m1b[:], m2a[:, :, 56:72])

                with tc.tile_pool(name="paqps", bufs=1, space="PSUM") as paqps:
                    warm_ps = paqps.tile(
                        [128, 512], f32, name="warm_ps", tag="pv", bufs=1
                    )
                    with tc.high_priority():
                        for _ in range(10):
                            nc.tensor.matmul(
                                warm_ps[:, 0:128], warm_stat[:], warm_stat[:],
                                start=True, stop=True,
                            )

                    def one_transpose(pend, j):
                        n, qtl, ktl, qkb = pend
                        tp = paqps.tile(
                            [128, 128], bf16, name="tp", tag="tp", bufs=1
                        )
                        if j < 4:
                            src, dst = qtl[:, 128 * j : 128 * (j + 1)], qTa[j]
                        elif j < 8:
                            src, dst = ktl[:, 128 * (j - 4) : 128 * (j - 3)], kTa[j - 4]
                        else:
                            src, dst = qkb[:], qkbT
                        nc.tensor.transpose(tp[:], src, ident_bf[:])
                        if j % 2:
                            nc.scalar.copy(dst[:, n * 128 : (n + 1) * 128], tp[:])
                        else:
                            nc.vector.tensor_copy(
                                dst[:, n * 128 : (n + 1) * 128], tp[:]
                            )

                    pend = None
                    for n in range(NT):
                        ps_q = paqps.tile(
                            [128, 512], f32, name="ps_q", tag="psq", bufs=2
                        )
                        ps_k = paqps.tile(
                            [128, 512], f32, name="ps_k", tag="psk", bufs=2
                        )
                        ps_b = paqps.tile(
                            [128, 192], f32, name="ps_b", tag="psb", bufs=2
                        )
                        ps_v = paqps.tile(
                            [128, 512], f32, name="ps_v", tag="pv", bufs=1
                        )
                        for k in range(KC):
                            st, sp = k == 0, k == KC - 1
                            lhs = xtiles[k][:, n * 128 : (n + 1) * 128]
                            nc.tensor.matmul(
                                ps_q[:], lhs, wb_tiles[k][:, 0:512],
                                start=st, stop=sp,
                            )
                            nc.tensor.matmul(
                                ps_k[:], lhs, wb_tiles[k][:, 512:1024],
                                start=st, stop=sp,
                            )
                            nc.tensor.matmul(
                                ps_b[:], lhs, wb_tiles[k][:, 1024:1216],
                                start=st, stop=sp,
                            )
                            nc.tensor.matmul(
                                ps_v[:], lhs, wb_tiles[k][:, 1216:WB],
                                start=st, stop=sp,
                            )
                            if pend is not None:
                                one_transpose(pend, k)
                        v3 = vt[n].rearrange("p (h e) -> p h e", h=HL)
                        c3, s3 = trig3(cos_sb, n), trig3(snp_sb, n)
                        # scalar engine: DVE is saturated by rope in the
                        # steady state and was backpressuring the v matmuls
                        nc.scalar.copy(
                            v3[:, :, 128:HD],
                            ps_b[:, 128:192].rearrange("p (h e) -> p h e", h=HL),
                        )
                        nc.scalar.copy(
                            v3[:, :, 0:128],
                            ps_v.rearrange("p (h e) -> p h e", h=HL),
                        )
                        nc.gpsimd.memset(v3[:, :, HD : HD + 1], 1.0)
                        qtl = pa.tile([128, 512], bf16, name="qtl", tag="qtl", bufs=2)
                        ktl = pa.tile([128, 512], bf16, name="ktl", tag="ktl", bufs=2)
                        qkb = pa.tile([128, 128], bf16, name="qkb", tag="qkb", bufs=2)
                        m1q = pa.tile([128, 576], bf16, name="m1q", tag="m1q", bufs=2)
                        m2q = pa.tile([128, 576], bf16, name="m2q", tag="m2q", bufs=2)
                        m1k = pa.tile([128, 576], bf16, name="m1k", tag="m1k", bufs=2)
                        m2k = pa.tile([128, 576], bf16, name="m2k", tag="m2k", bufs=2)
                        rope_b(
                            ps_b[:, 0:64].rearrange("p (h e) -> p h e", h=HL),
                            m1q, m2q, c3, s3,
                        )
                        rope_a(
                            ps_q, m1q, m2q, qtl,
                            qkb[:, 0:64].rearrange("p (h e) -> p h e", h=HL),
                            c3, s3,
                        )
                        rope_b(
                            ps_b[:, 64:128].rearrange("p (h e) -> p h e", h=HL),
                            m1k, m2k, c3, s3,
                        )
                        rope_a(
                            ps_k, m1k, m2k, ktl,
                            qkb[:, 64:128].rearrange("p (h e) -> p h e", h=HL),
                            c3, s3,
                        )
                        pend = (n, qtl, ktl, qkb)
                    for j in range(9):
                        one_transpose(pend, j)

                # replicate the b-rows to all four 32-row groups (16 data
                # rows each; the other 16 rows were pre-zeroed), in two
                # column halves so the first half's DMAs issue as soon as
                # transposes n<=7 are done (subtile deps)
                for half in range(2):
                    cl, cr = half * 1024, (half + 1) * 1024
                    for hh in range(HL):
                        for j in range(4):
                            nc.sync.dma_start(
                                qTBr[hh][32 * j : 32 * j + 16, cl:cr],
                                qkbT[16 * hh : 16 * hh + 16, cl:cr],
                            )
                            nc.sync.dma_start(
                                kTBr[hh][32 * j : 32 * j + 16, cl:cr],
                                qkbT[64 + 16 * hh : 64 + 16 * hh + 16, cl:cr],
                            )

            # ---------------- Phase B: attention --------------------------
            # v3: 512-wide q items (qb, h, kt).  Per item: one K=32 b-tail
            # (4 consecutive kts' tails issued together on the 4 distinct PE
            # row-groups -> concurrent in the array, ~1/4 the wall cost) +
            # one K=128 main (stop) + exp + 4 PV matmuls.  The exp is split:
            # cols 0:EXS on Scalar (exact ACT Exp), cols EXS:512 on DVE via a
            # Schraudolph bitcast (int16(x*a+b) viewed as bf16, one
            # tensor_scalar op) so neither engine paces the ~660ns/item PE
            # stream.  PSUM: sps 5x1 bank + O1 (ql 0-2) 2x1 + O2 (ql 3 +
            # denom) 1x1 = 8 banks.  O2 is normalized first at head
            # boundaries; the next head's ql3 PV arrives ~760ns later.
            with tc.tile_pool(name="pb", bufs=1) as pb:
                ot = [
                    pb.tile([128, DV], bf16, name=f"o{t}", tag=f"o{t}")
                    for t in range(NT)
                ]
                # phase C's Wo chunks: issue the DMAs now so they stream in
                # during phase B instead of gating the first final matmuls
                wo_tiles = []
                for k in range(5):
                    rows = 128 if k < 4 else 64
                    wot_ = pb.tile([128, D], bf16, name=f"wo{k}", tag=f"wo{k}")
                    nc.sync.dma_start(
                        wot_[0:rows, :], woT[k * 128 : k * 128 + rows, :]
                    )
                    wo_tiles.append(wot_)
                with tc.tile_pool(name="pbps", bufs=1, space="PSUM") as pbps:
                    HD1 = HD + 1
                    QB = 4              # 512-wide q blocks
                    NITEM = QB * HL * NT
                    LOOK = 3            # PV lags scores by 3 items
                    EXS = 256           # scalar-exp columns per 512
                    # Schraudolph bf16 exp: bits = int16(x*EXA + EXB)
                    EXA = SCALE * (2.0 ** 7) / float(np.log(2.0))
                    EXB = 127.0 * 128.0 - 7.5
                    i16 = dt.int16
                    Alu = mybir.AluOpType

                    def decode(idx):
                        qb, rem = divmod(idx, HL * NT)
                        h, kt = divmod(rem, NT)
                        return qb, h, kt

                    sps_pend = {}

                    def stage_tails(s):
                        # 4 consecutive kts' b-tails -> 4 distinct row-groups
                        qb, h, kt0 = decode(s)
                        for j in range(4):
                            kt = kt0 + j
                            rg = kt % 4
                            sps = pbps.tile(
                                [128, 512], f32, name="sps", tag="sc", bufs=6
                            )
                            nc.tensor.matmul(
                                sps[:],
                                kTBr[h][
                                    32 * rg : 32 * rg + 32,
                                    kt * 128 : (kt + 1) * 128,
                                ],
                                qTBr[h][
                                    32 * rg : 32 * rg + 32,
                                    qb * 512 : (qb + 1) * 512,
                                ],
                                start=True,
                                stop=False,
                                tile_position=(32 * rg, 0),
                            )
                            sps_pend[s + j] = sps

                    def stage_main(s):
                        qb, h, kt = decode(s)
                        sps = sps_pend.pop(s)
                        nc.tensor.matmul(
                            sps[:],
                            kTa[h][:, kt * 128 : (kt + 1) * 128],
                            qTa[h][:, qb * 512 : (qb + 1) * 512],
                            start=False,
                            stop=True,
                        )
                        E = pb.tile([128, 512], bf16, name="E", tag="E", bufs=6)
                        nc.scalar.activation(
                            E[:, 0:EXS], sps[:, 0:EXS], AF.Exp, scale=SCALE
                        )
                        nc.vector.tensor_scalar(
                            E[:, EXS:512].bitcast(i16),
                            sps[:, EXS:512],
                            EXA,
                            EXB,
                            Alu.mult,
                            Alu.add,
                        )
                        return E

                    o_ps = None
                    eq = []
                    for s in range(NITEM + LOOK):
                        if s < NITEM:
                            if s % 4 == 0:
                                stage_tails(s)
                            eq.append(stage_main(s))
                        if s < LOOK:
                            continue
                        idx = s - LOOK
                        qb, h, kt = decode(idx)
                        if kt == 0:
                            # (ql0,ql1) and (ql2,ql3) pair up in two banks:
                            # a start=True PV clears its whole bank, so each
                            # new head's PV ql waits on only TWO normalizes
                            # of the previous head, not three
                            O1 = pbps.tile(
                                [128, 2 * HD1], f32, name="O1", tag="O1", bufs=1
                            )
                            O2 = pbps.tile(
                                [128, 2 * HD1], f32, name="O2", tag="O2", bufs=1
                            )
                            o_ps = [
                                O1[:, 0:HD1], O1[:, HD1 : 2 * HD1],
                                O2[:, 0:HD1], O2[:, HD1 : 2 * HD1],
                            ]
                        E = eq.pop(0)
                        for ql in range(4):
                            st = kt == 0 and ql in (0, 2)
                            sp = kt == NT - 1 and ql in (1, 3)
                            nc.tensor.matmul(
                                o_ps[ql][:],
                                E[:, ql * 128 : (ql + 1) * 128],
                                vt[kt][:, HD1 * h : HD1 * (h + 1)],
                                start=st,
                                stop=sp,
                            )
                        if kt == NT - 1:
                            # accumulators are single-buffered: the next
                            # head's bank-clearing PV stalls on BOTH norms of
                            # that bank.  Batch the reciprocals (one [128,2]
                            # op per bank), then O1's norms on DVE and O2's
                            # on Scalar, all at high priority so they beat
                            # the queued exps of the in-flight score stream.
                            with tc.high_priority():
                                dsts = [
                                    ot[4 * qb + ql][:, HD * h : HD * (h + 1)]
                                    for ql in range(4)
                                ]
                                rA = pb.tile([128, 2], f32, name="rA", tag="rA", bufs=2)
                                rB = pb.tile([128, 2], f32, name="rB", tag="rB", bufs=2)
                                nc.vector.reciprocal_approx_fast(
                                    rA.rearrange("p (q o) -> p q o", q=2),
                                    O1.rearrange("p (q e) -> p q e", q=2)[
                                        :, :, HD : HD + 1
                                    ],
                                )
                                nc.vector.reciprocal_approx_fast(
                                    rB.rearrange("p (q o) -> p q o", q=2),
                                    O2.rearrange("p (q e) -> p q e", q=2)[
                                        :, :, HD : HD + 1
                                    ],
                                )
                                nc.scalar.activation(
                                    dsts[2], o_ps[2][:, 0:HD], AF.Copy,
                                    scale=rB[:, 0:1],
                                )
                                nc.scalar.activation(
                                    dsts[3], o_ps[3][:, 0:HD], AF.Copy,
                                    scale=rB[:, 1:2],
                                )
                                nc.vector.tensor_scalar_mul(
                                    dsts[0], o_ps[0][:, 0:HD], rA[:, 0:1]
                                )
                                nc.vector.tensor_scalar_mul(
                                    dsts[1], o_ps[1][:, 0:HD], rA[:, 1:2]
                                )

                # ---------------- Phase C: o^T + final projection ----------
                oTa = [
                    pb.tile([128, T], bf16, name=f"oTa{j}", tag=f"oTa{j}")
                    for j in range(4)
                ]
                oTb = pb.tile([64, T], bf16, name="oTb", tag="oTb")
                with tc.tile_pool(name="pcps", bufs=1, space="PSUM") as pcps:

                    def o_transp(t):
                        for j in range(4):
                            tp = pcps.tile(
                                [128, 128], bf16, name="tpo", tag="otp", bufs=3
                            )
                            nc.tensor.transpose(
                                tp[:],
                                ot[t][:, 128 * j : 128 * (j + 1)],
                                ident_bf[:],
                            )
                            nc.vector.tensor_copy(
                                oTa[j][:, t * 128 : (t + 1) * 128], tp[:]
                            )
                        tpb = pcps.tile([64, 128], bf16, name="tpb", tag="otp", bufs=3)
                        nc.tensor.transpose(
                            tpb[:],
                            ot[t][:, 512:DV],
                            ident_bf[:],
                        )
                        nc.vector.tensor_copy(
                            oTb[:, t * 128 : (t + 1) * 128], tpb[:]
                        )

                    def final(t):
                        fps = [
                            pcps.tile(
                                [128, 384], f32, name=f"fps{j3}", tag=f"f{j3}",
                                bufs=(2 if j3 < 2 else 1),
                            )
                            for j3 in range(3)
                        ]
                        # k-outer / j3-inner: the 3 matmuls of each k share
                        # the o^T stationary, hiding its LDWEIGHTS
                        for k in range(5):
                            rows = 128 if k < 4 else 64
                            lhsT = (
                                oTa[k][:, t * 128 : (t + 1) * 128]
                                if k < 4
                                else oTb[:, t * 128 : (t + 1) * 128]
                            )
                            for j3 in range(3):
                                nc.tensor.matmul(
                                    fps[j3][:],
                                    lhsT,
                                    wo_tiles[k][0:rows, 384 * j3 : 384 * (j3 + 1)],
                                    start=(k == 0),
                                    stop=(k == 4),
                                )
                        for j3 in range(3):
                            fout = pb.tile(
                                [128, 384], f32, name="fout", tag="fout", bufs=10
                            )
                            if (t * 3 + j3) % 2 == 1:
                                nc.vector.tensor_copy(fout[:], fps[j3][:])
                            else:
                                nc.scalar.copy(fout[:], fps[j3][:])
                            nc.sync.dma_start(
                                out[
                                    t * 128 : (t + 1) * 128,
                                    384 * j3 : 384 * (j3 + 1),
                                ],
                                fout[:],
                            )

                    o_transp(0)
                    for t in range(NT):
                        if t + 1 < NT:
                            o_transp(t + 1)
                        final(t)

    nc.compile()
    return nc


def get_nc(debug=False):
    key = bool(debug)
    if key not in _NC_CACHE:
        _NC_CACHE[key] = _build(debug)
    return _NC_CACHE[key]


def make_in_maps(x, cos, sin, Wq, Wk, Wv, Wo):
    import ml_dtypes

    x = np.asarray(x, np.float32)
    cos = np.asarray(cos, np.float32)
    sin = np.asarray(sin, np.float32)
    Wq, Wk, Wv, Wo = (np.asarray(w, np.float32) for w in (Wq, Wk, Wv, Wo))
    cos_bf = cos.astype(ml_dtypes.bfloat16)
    # sign-folded, partner-permuted sin: snP[t,i] = sin[t,(i+72)%144] * s,
    # s = +1 for i<72, -1 for i>=72; makes every rope combine a plain add
    snp = sin[:, (np.arange(HD) + 72) % HD].copy()
    snp[:, 72:] *= -1.0
    snp_bf = snp.astype(ml_dtypes.bfloat16)

    in_maps = []
    for c in range(NCORES):
        b, hg = divmod(c, 2)
        heads = [HL * hg + i for i in range(HL)]

        def w_merged(Wq_, Wk_, Wv_):
            # rows: [q-a 4x128 | k-a 4x128 | q-b 4x16 | k-b 4x16 | v-b 4x16
            #        | v-a 4x128]
            Wsel = np.zeros((WB, D), np.float32)
            for i, g in enumerate(heads):
                a, bb = 144 * g, 144 * g + 128
                Wsel[128 * i : 128 * i + 128] = Wq_[a : a + 128]
                Wsel[512 + 128 * i : 512 + 128 * i + 128] = Wk_[a : a + 128]
                Wsel[1024 + 16 * i : 1024 + 16 * i + 16] = Wq_[bb : bb + 16]
                Wsel[1088 + 16 * i : 1088 + 16 * i + 16] = Wk_[bb : bb + 16]
                Wsel[1152 + 16 * i : 1152 + 16 * i + 16] = Wv_[bb : bb + 16]
                Wsel[1216 + 128 * i : 1216 + 128 * i + 128] = Wv_[a : a + 128]
            return np.ascontiguousarray(Wsel.T)

        wo_sel = np.concatenate([Wo[:, 144 * g : 144 * g + 144] for g in heads], 1)
        in_maps.append(
            {
                "xT": np.ascontiguousarray(x[b].T).astype(ml_dtypes.bfloat16),
                "wbT": w_merged(Wq, Wk, Wv).astype(ml_dtypes.bfloat16),
                "woT": np.ascontiguousarray(wo_sel.T).astype(ml_dtypes.bfloat16),
                "cosN": cos_bf,
                "snPN": snp_bf,
                "identB": np.eye(128, dtype=ml_dtypes.bfloat16),
            }
        )
    return in_maps


def kernel(x, cos, sin, Wq, Wk, Wv, Wo, _trace=False, _trace_kwargs=None):
    from concourse.bass_utils import run_bass_kernel_spmd

    nc = get_nc()
    in_maps = make_in_maps(x, cos, sin, Wq, Wk, Wv, Wo)
    res = run_bass_kernel_spmd(
        nc,
        in_maps,
        list(range(NCORES)),
        trace=_trace,
        **(_trace_kwargs or {}),
    )
    parts = [res.results[c]["out"] for c in range(NCORES)]
    outb = np.stack([parts[2 * b] + parts[2 * b + 1] for b in range(B)])
    if _trace:
        kernel.last_results = res
    return outb.astype(np.float32)

